# revision 50
# baseline (speedup 1.0000x reference)
"""Trainium2 Bass kernel for nn_AttentionOperator_43069932044621.

Math: the reference is rank-1 attention on scalar tokens:
  q = x[:,None]*w_q ; k = x[:,None]*w_k ; v = x[:,None]*w_v
  scores[b,n,m] = (q.k)/sqrt(D) = c * x[b,n] * x[b,m],  c = (w_q.w_k)/sqrt(16)/TAU
  out[b,n] = s * (sum_m x_m e^{a_n x_m}) / (sum_m e^{a_n x_m}),
             a_n = c*x[b,n],  s = (w_v.w_out)
Scores are in [-4, 4] for this data, so no softmax max-subtraction is needed
(verified: fp32 without stabilization matches reference to ~6e-7).

Sharding: 8 cores = 4 batches x 2 query-halves. Each core holds its batch's
full key row x[b] (4096) and computes 2048 queries x 4096 keys.

Device algorithm per core (key-partition layout):
  - compute c,s on device from w_q/w_k/w_v/w_out (replicated tiny dots)
  - broadcast queries to all 128 partitions via PE (ones[1,128]^T @ xq)
  - for each key tile j (32 tiles of 128 keys):
      E_j[p,q] = exp(cx_k[p,j] * xq[q])            (one ACT instr, [128,2048])
      psum[0,q] += sum_p 1 * E_j[p,q]   (den)       (PE matmul, stationary [128,2])
      psum[1,q] += sum_p s*x_k * E_j[p,q] (num*s)
  - out[q] = psum[1,q] / psum[0,q]
"""

import os
import numpy as np
from contextlib import ExitStack

import concourse.bass as bass
import concourse.tile as tile
from concourse import bacc, mybir
from concourse.bass_utils import run_bass_kernel_spmd

F32 = mybir.dt.float32
F32R = mybir.dt.float32r

B = 4
N = 4096
NCORES = 8
QPC = N // (NCORES // B)      # 2048 queries per core
KT = N // 128                 # 32 key tiles
CHUNK = 512                   # matmul moving free-dim chunk (one PSUM bank)
NCHUNK = QPC // CHUNK         # 4

# matmul dtype for the reduction over keys:
#   "f32r": full speed (1 cyc/col), ~1e-4 worst-case relative error
#   "f32" : 4x slower on PE, exact fp32
MM_DTYPE = os.environ.get("ATTN_MM_DTYPE", "f32")

# which kernel: "moment4" (Taylor-series in a, O(N*J) work, optimized),
# older variants moment/moment2/moment3, or "brute" (O(N^2) exps, exact
# for any score range -- used as fallback if scores exceed Taylor range)
KERNEL = os.environ.get("ATTN_KERNEL", "moment4")
J = int(os.environ.get("ATTN_J", "14"))  # Taylor degree; validated 1.5e-6 at J=14
DUAL = os.environ.get("ATTN_DUAL", "1") == "1"  # den+num Horner in one [128,32] tile
# max |score| the J=14 Taylor path is trusted for; beyond -> brute fallback
T_GUARD = 4.2


def _build_brute(nc):
    """Emit the SPMD program (same for every core) into nc."""
    xq = nc.dram_tensor("xq", [1, QPC], F32, kind="ExternalInput").ap()
    xk = nc.dram_tensor("xk", [128, KT], F32, kind="ExternalInput").ap()
    w = nc.dram_tensor("w", [1, 64], F32, kind="ExternalInput").ap()
    scratch = nc.dram_tensor("scratch", [2, QPC], F32).ap()
    out = nc.dram_tensor("out", [128, QPC // 128], F32, kind="ExternalOutput").ap()

    with tile.TileContext(nc) as tc, ExitStack() as ctx:
        sb = ctx.enter_context(tc.tile_pool(name="sb", bufs=1))
        epool = ctx.enter_context(tc.tile_pool(name="epool", bufs=3))
        psq = ctx.enter_context(tc.tile_pool(name="psq", bufs=1, space="PSUM"))
        psa = ctx.enter_context(tc.tile_pool(name="psa", bufs=1, space="PSUM"))

        # ---- load inputs ----
        w_bc = sb.tile([128, 64], F32)
        w_bcast_ap = bass.AP(tensor=w.tensor, offset=w.offset,
                             ap=[[0, 128]] + list(w.ap[1:]))
        nc.sync.dma_start(out=w_bc, in_=w_bcast_ap)
        xq_sb = sb.tile([1, QPC], F32)
        nc.sync.dma_start(out=xq_sb, in_=xq)
        xk_sb = sb.tile([128, KT], F32)
        nc.sync.dma_start(out=xk_sb, in_=xk)

        # ---- c = 0.25*dot(wq,wk), s = dot(wv,wout), replicated on all partitions
        prod = sb.tile([128, 32], F32)
        nc.vector.tensor_mul(prod[:, 0:16], w_bc[:, 0:16], w_bc[:, 16:32])
        nc.vector.tensor_mul(prod[:, 16:32], w_bc[:, 32:48], w_bc[:, 48:64])
        cs = sb.tile([128, 2], F32)
        nc.vector.reduce_sum(cs[:, 0:1], prod[:, 0:16], axis=mybir.AxisListType.X)
        nc.vector.reduce_sum(cs[:, 1:2], prod[:, 16:32], axis=mybir.AxisListType.X)
        nc.scalar.mul(out=cs[:, 0:1], in_=cs[:, 0:1], mul=0.25)

        # ---- cx_k and stationary (1 | s*x_k) interleaved columns ----
        mm_dt = F32R if MM_DTYPE == "f32r" else F32
        cxk = sb.tile([128, KT], F32)
        nc.vector.tensor_scalar_mul(out=cxk, in0=xk_sb, scalar1=cs[:, 0:1])
        stat = sb.tile([128, 2 * KT], mm_dt)
        stat3 = stat.rearrange("p (j t) -> p j t", t=2)
        xk3 = xk_sb.rearrange("p (j t) -> p j t", t=1)
        # ones in even columns: (xk*0)+1 — memset can't write f32r
        nc.vector.tensor_scalar(out=stat3[:, :, 0:1], in0=xk3,
                                scalar1=0.0, scalar2=1.0,
                                op0=mybir.AluOpType.mult,
                                op1=mybir.AluOpType.add)
        nc.vector.tensor_scalar_mul(out=stat3[:, :, 1:2], in0=xk3,
                                    scalar1=cs[:, 1:2])

        # ---- broadcast queries to all partitions (PE with ones stationary) ----
        ones_row = sb.tile([1, 128], F32)
        nc.vector.memset(ones_row, 1.0)
        ps_q = psq.tile([128, QPC], F32)
        for cix in range(NCHUNK):
            sl = slice(cix * CHUNK, (cix + 1) * CHUNK)
            nc.tensor.matmul(ps_q[:, sl], lhsT=ones_row,
                             rhs=xq_sb[:, sl], start=True, stop=True)

        # ---- main loop over key tiles ----
        ps_acc = psa.tile([2, QPC], F32)
        for j in range(KT):
            e = epool.tile([128, QPC], mm_dt, tag="e")
            nc.scalar.activation(out=e, in_=ps_q,
                                 func=mybir.ActivationFunctionType.Exp,
                                 scale=cxk[:, j:j + 1])
            for cix in range(NCHUNK):
                sl = slice(cix * CHUNK, (cix + 1) * CHUNK)
                nc.tensor.matmul(ps_acc[:, sl],
                                 lhsT=stat[:, 2 * j:2 * j + 2],
                                 rhs=e[:, sl],
                                 start=(j == 0), stop=(j == KT - 1),
                                 skip_group_check=True)

        # ---- tail: out = psum[1]/psum[0] ----
        cp = sb.tile([2, QPC], F32)
        nc.scalar.copy(out=cp, in_=ps_acc)
        # bounce through DRAM to reshape [2,2048] -> 2 x [128,16]
        nc.sync.dma_start(out=scratch, in_=cp)
        den_t = sb.tile([128, QPC // 128], F32)
        num_t = sb.tile([128, QPC // 128], F32)
        sc128 = scratch.rearrange("r (p f) -> r p f", p=128)
        nc.sync.dma_start(out=den_t, in_=sc128[0])
        nc.sync.dma_start(out=num_t, in_=sc128[1])
        recip = sb.tile([128, QPC // 128], F32)
        nc.vector.reciprocal(out=recip, in_=den_t)
        out_t = sb.tile([128, QPC // 128], F32)
        nc.vector.tensor_mul(out_t, num_t, recip)
        nc.sync.dma_start(out=out, in_=out_t)

    return nc


def _build_moment_v2(nc):
    """Optimized moment/Taylor kernel (see _build_moment docstring).

    Changes vs v1: two fused input DMAs, no ScalarE (no ACT table load),
    moments reduced via PE matmul + one wide DVE reduce instead of per-power
    reduces, den+num Horner fused in one [128, 2*QT] tile using a stride-0
    coefficient view, J=16.
    """
    f32 = F32
    QT = QPC // 128          # 16 queries per partition
    J1 = J + 1               # powers x^1..x^{J+1} -> J+1 slices
    NW = 2 * J + 4           # D0..DJ | N0..NJ | c | s
    xin = nc.dram_tensor("xin", [128, KT + QT], f32, kind="ExternalInput").ap()
    win = nc.dram_tensor("win", [1, 96], f32, kind="ExternalInput").ap()
    out = nc.dram_tensor("out", [128, QT], f32, kind="ExternalOutput").ap()

    with tile.TileContext(nc) as tc, ExitStack() as ctx:
        sb = ctx.enter_context(tc.tile_pool(name="sb", bufs=1))
        ps = ctx.enter_context(tc.tile_pool(name="ps", bufs=1, space="PSUM"))

        dma_in = nc.scalar if os.environ.get("ATTN_DMA", "scalar") == "scalar" else nc.sync
        xin_sb = sb.tile([128, KT + QT], f32)
        dma_in.dma_start(out=xin_sb, in_=xin)
        win_sb = sb.tile([1, 96], f32)
        dma_in.dma_start(out=win_sb, in_=win)
        xk = xin_sb[:, 0:KT]
        xq_t = xin_sb[:, KT:KT + QT]
        fact = win_sb[:, 64:96]

        # c = 0.25*dot(wq,wk), s = dot(wv,wout) on partition 0 (DVE only)
        prod = sb.tile([1, 32], f32)
        nc.vector.tensor_mul(prod[:, 0:16], win_sb[:, 0:16], win_sb[:, 16:32])
        nc.vector.tensor_mul(prod[:, 16:32], win_sb[:, 32:48], win_sb[:, 48:64])
        cs = sb.tile([1, 2], f32)
        nc.vector.reduce_sum(cs[:, 0:1], prod[:, 0:16], axis=mybir.AxisListType.X)
        nc.vector.reduce_sum(cs[:, 1:2], prod[:, 16:32], axis=mybir.AxisListType.X)
        nc.vector.tensor_scalar_mul(out=cs[:, 0:1], in0=cs[:, 0:1], scalar1=0.25)

        # ---- phase 1: powers x^1..x^{J+1} as slices of one tile ----
        u_all = sb.tile([128, J1 * KT], f32)
        nc.vector.tensor_copy(u_all[:, 0:KT], xk)
        for i in range(1, J1):
            nc.vector.tensor_mul(u_all[:, i * KT:(i + 1) * KT],
                                 u_all[:, (i - 1) * KT:i * KT], xk)

        # cross-partition sums via PE; then one wide free-reduce
        ones_col = sb.tile([128, 1], f32)
        nc.vector.memset(ones_col, 1.0)
        ps_m = ps.tile([1, J1 * KT], f32)
        for lo in range(0, J1 * KT, 512):
            hi = min(lo + 512, J1 * KT)
            nc.tensor.matmul(ps_m[:, lo:hi], lhsT=ones_col,
                             rhs=u_all[:, lo:hi], start=True, stop=True)
        M = sb.tile([1, J1], f32)
        nc.vector.reduce_sum(M, ps_m.rearrange("o (j f) -> o j f", f=KT),
                             axis=mybir.AxisListType.X)

        # ---- coefficient row: D | N | c | s on partition 0 ----
        row = sb.tile([1, NW], f32)
        nc.vector.memset(row[:, 0:1], float(N))           # D_0 = M_0 = N keys
        nc.vector.tensor_mul(row[:, 1:J + 1], M[:, 0:J], fact[:, 1:J + 1])
        nc.vector.tensor_mul(row[:, J + 1:2 * J + 2], M[:, 0:J + 1],
                             fact[:, 0:J + 1])
        nc.vector.tensor_copy(row[:, 2 * J + 2:NW], cs)

        # ---- broadcast to all partitions ----
        ones_row = sb.tile([1, 128], f32)
        nc.vector.memset(ones_row, 1.0)
        ps_bc = ps.tile([128, NW], f32)
        nc.tensor.matmul(ps_bc, lhsT=ones_row, rhs=row, start=True, stop=True)
        DN = sb.tile([128, NW], f32)
        nc.vector.tensor_copy(DN, ps_bc)

        def dn_view(j):
            # [128, 2, QT] view: [:, 0, :] = D_j broadcast, [:, 1, :] = N_j
            return bass.AP(tensor=DN.tensor, offset=DN.offset + j,
                           ap=[list(DN.ap[0]), [J + 1, 2], [0, QT]])

        # ---- phase 2: fused dual Horner at a = c*xq ----
        a_dup = sb.tile([128, 2 * QT], f32)
        nc.vector.tensor_scalar_mul(out=a_dup[:, 0:QT], in0=xq_t,
                                    scalar1=DN[:, 2 * J + 2:2 * J + 3])
        nc.vector.tensor_copy(a_dup[:, QT:2 * QT], a_dup[:, 0:QT])
        h = sb.tile([128, 2 * QT], f32)
        h3 = h.rearrange("p (t f) -> p t f", t=2)
        if DUAL:
            nc.vector.tensor_copy(h3, dn_view(J))
            for j in range(J - 1, -1, -1):
                nc.vector.tensor_mul(h, h, a_dup)
                nc.vector.tensor_add(h3, h3, dn_view(j))
        else:
            nc.vector.tensor_scalar(out=h[:, 0:QT], in0=a_dup[:, 0:QT],
                                    scalar1=0.0, scalar2=DN[:, J:J + 1],
                                    op0=mybir.AluOpType.mult,
                                    op1=mybir.AluOpType.add)
            nc.vector.tensor_scalar(out=h[:, QT:2 * QT], in0=a_dup[:, 0:QT],
                                    scalar1=0.0, scalar2=DN[:, 2 * J + 1:2 * J + 2],
                                    op0=mybir.AluOpType.mult,
                                    op1=mybir.AluOpType.add)
            for j in range(J - 1, -1, -1):
                nc.vector.tensor_mul(h, h, a_dup)
                nc.vector.tensor_scalar_add(out=h[:, 0:QT], in0=h[:, 0:QT],
                                            scalar1=DN[:, j:j + 1])
                nc.vector.tensor_scalar_add(out=h[:, QT:2 * QT],
                                            in0=h[:, QT:2 * QT],
                                            scalar1=DN[:, J + 1 + j:J + 2 + j])

        # ---- out = s * num/den ----
        r = sb.tile([128, QT], f32)
        nc.vector.reciprocal(out=r, in_=h[:, 0:QT])
        out_t = sb.tile([128, QT], f32)
        nc.vector.tensor_mul(out_t, h[:, QT:2 * QT], r)
        nc.vector.tensor_scalar_mul(out=out_t, in0=out_t,
                                    scalar1=DN[:, 2 * J + 3:NW])
        nc.sync.dma_start(out=out, in_=out_t)

    return nc


def _build_moment_v3(nc):
    """v3: ping-pong power chain, per-j reduces, tiny PE reduction matmul,
    Estrin evaluation (J even), DVE-only compute + 2 tiny matmuls."""
    f32 = F32
    QT = QPC // 128
    NW = 2 * J + 4
    PH1 = os.environ.get("ATTN_PH1", "pp")  # pp | wide
    assert J % 2 == 0
    xin = nc.dram_tensor("xin", [128, KT + QT], f32, kind="ExternalInput").ap()
    win = nc.dram_tensor("win", [1, 96], f32, kind="ExternalInput").ap()
    out = nc.dram_tensor("out", [128, QT], f32, kind="ExternalOutput").ap()

    with tile.TileContext(nc) as tc, ExitStack() as ctx:
        sb = ctx.enter_context(tc.tile_pool(name="sb", bufs=1))
        up = ctx.enter_context(tc.tile_pool(name="up", bufs=2))
        ps = ctx.enter_context(tc.tile_pool(name="ps", bufs=1, space="PSUM"))

        xin_sb = sb.tile([128, KT + QT], f32)
        nc.scalar.dma_start(out=xin_sb, in_=xin)
        win_sb = sb.tile([1, 96], f32)
        nc.scalar.dma_start(out=win_sb, in_=win)
        xk = xin_sb[:, 0:KT]
        xq_t = xin_sb[:, KT:KT + QT]
        fact = win_sb[:, 64:96]

        # ---- phase 1: moments M_1..M_{J+1}; chain first on DVE ----
        NM = J + 1
        U = sb.tile([128, NM], f32)
        if PH1 == "pp":
            nc.vector.reduce_sum(U[:, 0:1], xk, axis=mybir.AxisListType.X)
            uprev = xk
            for i in range(1, NM):
                u = up.tile([128, KT], f32, tag="u")
                nc.vector.tensor_mul(u, uprev, xk)
                nc.vector.reduce_sum(U[:, i:i + 1], u, axis=mybir.AxisListType.X)
                uprev = u
        else:
            u_all = sb.tile([128, NM * KT], f32)
            nc.vector.tensor_copy(u_all[:, 0:KT], xk)
            for i in range(1, NM):
                nc.vector.tensor_mul(u_all[:, i * KT:(i + 1) * KT],
                                     u_all[:, (i - 1) * KT:i * KT], xk)
            nc.vector.reduce_sum(U, u_all.rearrange("p (j f) -> p j f", f=KT),
                                 axis=mybir.AxisListType.X)

        # cs dots overlap the PE reduction below
        prod = sb.tile([1, 32], f32)
        nc.vector.tensor_mul(prod[:, 0:16], win_sb[:, 0:16], win_sb[:, 16:32])
        nc.vector.tensor_mul(prod[:, 16:32], win_sb[:, 32:48], win_sb[:, 48:64])
        cs = sb.tile([1, 2], f32)
        nc.vector.reduce_sum(cs[:, 0:1], prod[:, 0:16], axis=mybir.AxisListType.X)
        nc.vector.reduce_sum(cs[:, 1:2], prod[:, 16:32], axis=mybir.AxisListType.X)
        nc.vector.tensor_scalar_mul(out=cs[:, 0:1], in0=cs[:, 0:1], scalar1=0.25)

        ones_col = sb.tile([128, 1], f32)
        nc.vector.memset(ones_col, 1.0)
        ps_m = ps.tile([1, NM], f32)
        nc.tensor.matmul(ps_m, lhsT=ones_col, rhs=U, start=True, stop=True)

        # ---- coefficient row ----
        row = sb.tile([1, NW], f32)
        nc.vector.memset(row[:, 0:1], float(N))
        nc.vector.tensor_mul(row[:, 1:J + 1], ps_m[:, 0:J], fact[:, 1:J + 1])
        nc.vector.tensor_mul(row[:, J + 1:2 * J + 2], ps_m[:, 0:J + 1],
                             fact[:, 0:J + 1])
        nc.vector.tensor_copy(row[:, 2 * J + 2:NW], cs)

        ones_row = sb.tile([1, 128], f32)
        nc.vector.memset(ones_row, 1.0)
        ps_bc = ps.tile([128, NW], f32)
        nc.tensor.matmul(ps_bc, lhsT=ones_row, rhs=row, start=True, stop=True)
        DN = sb.tile([128, NW], f32)
        nc.vector.tensor_copy(DN, ps_bc)

        # ---- phase 2: Estrin at a = c*xq for den and num ----
        a_t = sb.tile([128, QT], f32)
        nc.vector.tensor_scalar_mul(out=a_t, in0=xq_t,
                                    scalar1=DN[:, 2 * J + 2:2 * J + 3])
        a2 = sb.tile([128, QT], f32)
        nc.vector.tensor_mul(a2, a_t, a_t)

        pp = ctx.enter_context(tc.tile_pool(name="pp", bufs=1))

        def estrin(coef_off, hname):
            # coefficients C_j at DN[:, coef_off + j]
            h = pp.tile([128, QT], f32, name=hname)
            nc.vector.tensor_scalar(out=h, in0=a_t, scalar1=0.0,
                                    scalar2=DN[:, coef_off + J:coef_off + J + 1],
                                    op0=mybir.AluOpType.mult,
                                    op1=mybir.AluOpType.add)
            ptiles = []
            for i in range(J // 2):
                p = pp.tile([128, QT], f32, name=f"{hname}_p{i}")
                nc.vector.tensor_scalar(
                    out=p, in0=a_t,
                    scalar1=DN[:, coef_off + 2 * i + 1:coef_off + 2 * i + 2],
                    scalar2=DN[:, coef_off + 2 * i:coef_off + 2 * i + 1],
                    op0=mybir.AluOpType.mult, op1=mybir.AluOpType.add)
                ptiles.append(p)
            for i in range(J // 2 - 1, -1, -1):
                nc.vector.tensor_mul(h, h, a2)
                nc.vector.tensor_add(h, h, ptiles[i])
            return h

        hd = estrin(0, "hd")
        hn = estrin(J + 1, "hn")

        r = sb.tile([128, QT], f32)
        nc.vector.reciprocal(out=r, in_=hd)
        out_t = sb.tile([128, QT], f32)
        nc.vector.tensor_mul(out_t, hn, r)
        nc.vector.tensor_scalar_mul(out=out_t, in0=out_t,
                                    scalar1=DN[:, 2 * J + 3:NW])
        nc.sync.dma_start(out=out, in_=out_t)

    return nc


def _col_d(j):
    """Column of D_j in the permuted coefficient row (see _build_moment_v4)."""
    if j == 0:
        return 0
    return 1 + (j - 1) // 2 if j % 2 == 1 else 8 + (j // 2 - 1)


def _col_n(j):
    return 15 + j // 2 if j % 2 == 0 else 23 + (j - 1) // 2


def _build_moment_v4(nc):
    """v4: power chain alternates between two tensors (x^p = x^{p-1} * x with
    odd powers in A, even in B) so no op reads the tensor it writes; moments
    land permuted and the factorial row comes from the host pre-permuted.
    Optionally offloads den-side Estrin pairs to the Scalar engine (ACT
    Identity = in*scale+bias with per-partition APs), warmed by a dummy
    activation at kernel start."""
    f32 = F32
    QT = QPC // 128
    assert J == 14, "v4 layout is hardcoded for J=14"
    NW = 32  # D0 | Dodd(7) | Deven(7) | Neven(8) | Nodd(7) | c | s
    ACT_PAIRS = os.environ.get("ATTN_ACT", "1") == "1"
    xin = nc.dram_tensor("xin", [128, KT + QT], f32, kind="ExternalInput").ap()
    win = nc.dram_tensor("win", [1, 96], f32, kind="ExternalInput").ap()
    out = nc.dram_tensor("out", [128, QT], f32, kind="ExternalOutput").ap()

    with tile.TileContext(nc) as tc, ExitStack() as ctx:
        sb = ctx.enter_context(tc.tile_pool(name="sb", bufs=1))
        ps = ctx.enter_context(tc.tile_pool(name="ps", bufs=1, space="PSUM"))

        xin_sb = sb.tile([128, KT + QT], f32)
        nc.scalar.dma_start(out=xin_sb, in_=xin)
        win_sb = sb.tile([1, 96], f32)
        nc.sync.dma_start(out=win_sb, in_=win)
        xk = xin_sb[:, 0:KT]
        xq_t = xin_sb[:, KT:KT + QT]

        if ACT_PAIRS:
            warm = sb.tile([1, 1], f32)
            nc.vector.memset(warm, 0.0)
            nc.scalar.add(out=warm, in_=warm, add=0.0)  # absorb ACT table load

        # ---- phase 1: odd powers x^3..x^15 on DVE, even x^2..x^14 on ACT ----
        SPLIT_CHAIN = os.environ.get("ATTN_SPLIT", "1") == "1" and ACT_PAIRS
        A = sb.tile([128, 7 * KT], f32)   # x^3, x^5, ..., x^15
        Bt = sb.tile([128, 7 * KT], f32)  # x^2, x^4, ..., x^14
        x2d = sb.tile([128, KT], f32)
        nc.vector.tensor_mul(x2d, xk, xk)
        nc.vector.tensor_mul(A[:, 0:KT], x2d, xk)            # x^3
        for i in range(1, 7):                                 # x^5..x^15
            nc.vector.tensor_mul(A[:, i * KT:(i + 1) * KT],
                                 A[:, (i - 1) * KT:i * KT], x2d)
        if SPLIT_CHAIN:
            sq = mybir.ActivationFunctionType.Square
            nc.scalar.activation(out=Bt[:, 0:KT], in_=xk, func=sq)          # x^2
            nc.scalar.activation(out=Bt[:, KT:2 * KT], in_=Bt[:, 0:KT], func=sq)   # x^4
            nc.scalar.activation(out=Bt[:, 2 * KT:3 * KT], in_=A[:, 0:KT], func=sq)  # x^6
            nc.scalar.activation(out=Bt[:, 3 * KT:4 * KT], in_=Bt[:, KT:2 * KT], func=sq)  # x^8
            nc.scalar.activation(out=Bt[:, 4 * KT:5 * KT], in_=A[:, KT:2 * KT], func=sq)   # x^10
            nc.scalar.activation(out=Bt[:, 5 * KT:6 * KT], in_=Bt[:, 2 * KT:3 * KT], func=sq)  # x^12
            nc.scalar.activation(out=Bt[:, 6 * KT:7 * KT], in_=A[:, 2 * KT:3 * KT], func=sq)   # x^14
        else:
            nc.vector.tensor_copy(Bt[:, 0:KT], x2d)           # x^2
            for i in range(1, 7):
                nc.vector.tensor_mul(Bt[:, i * KT:(i + 1) * KT],
                                     Bt[:, (i - 1) * KT:i * KT], x2d)
        U = sb.tile([128, 15], f32)
        nc.vector.reduce_sum(U[:, 0:1], xk, axis=mybir.AxisListType.X)
        nc.vector.reduce_sum(U[:, 1:8], A.rearrange("p (j f) -> p j f", f=KT),
                             axis=mybir.AxisListType.X)
        nc.vector.reduce_sum(U[:, 8:15], Bt.rearrange("p (j f) -> p j f", f=KT),
                             axis=mybir.AxisListType.X)

        # cs dots (overlap PE below)
        prod = sb.tile([1, 32], f32)
        nc.vector.tensor_mul(prod[:, 0:16], win_sb[:, 0:16], win_sb[:, 16:32])
        nc.vector.tensor_mul(prod[:, 16:32], win_sb[:, 32:48], win_sb[:, 48:64])
        cs = sb.tile([1, 2], f32)
        nc.vector.reduce_sum(cs[:, 0:1], prod[:, 0:16], axis=mybir.AxisListType.X)
        nc.vector.reduce_sum(cs[:, 1:2], prod[:, 16:32], axis=mybir.AxisListType.X)
        nc.vector.tensor_scalar_mul(out=cs[:, 0:1], in0=cs[:, 0:1], scalar1=0.25)

        ones_col = sb.tile([128, 1], f32)
        nc.vector.memset(ones_col, 1.0)
        ps_m = ps.tile([1, 15], f32)  # [M1,M3..M15, M2,M4..M14]
        nc.tensor.matmul(ps_m, lhsT=ones_col, rhs=U, start=True, stop=True)

        # ---- coefficient row (permuted layout) ----
        # win factors: 64: fDodd(7)=1/1!,1/3!..1/13!; 71: fDeven(7)=1/2!..1/14!
        #              78: fNeven(8)=1/0!,1/2!..1/14!; 86: fNodd(7)=1/1!..1/13!
        row = sb.tile([1, NW], f32)
        nc.vector.memset(row[:, 0:1], float(N))
        nc.vector.tensor_mul(row[:, 1:8], ps_m[:, 0:7], win_sb[:, 64:71])
        nc.vector.tensor_mul(row[:, 8:15], ps_m[:, 8:15], win_sb[:, 71:78])
        nc.vector.tensor_mul(row[:, 15:23], ps_m[:, 0:8], win_sb[:, 78:86])
        nc.vector.tensor_mul(row[:, 23:30], ps_m[:, 8:15], win_sb[:, 86:93])
        nc.vector.tensor_copy(row[:, 30:32], cs)

        ones_row = sb.tile([1, 128], f32)
        nc.vector.memset(ones_row, 1.0)
        ps_bc = ps.tile([128, NW], f32)
        nc.tensor.matmul(ps_bc, lhsT=ones_row, rhs=row, start=True, stop=True)
        DN = sb.tile([128, NW], f32)
        nc.vector.tensor_copy(DN, ps_bc)

        def dcol(j):
            return DN[:, _col_d(j):_col_d(j) + 1]

        def ncol(j):
            return DN[:, _col_n(j):_col_n(j) + 1]

        # ---- phase 2: Estrin; den pairs on ACT (parallel), num on DVE ----
        a_t = sb.tile([128, QT], f32)
        nc.vector.tensor_scalar_mul(out=a_t, in0=xq_t, scalar1=DN[:, 30:31])
        a2 = sb.tile([128, QT], f32)
        nc.vector.tensor_mul(a2, a_t, a_t)

        pp = ctx.enter_context(tc.tile_pool(name="pp", bufs=1))

        def make_pairs(col, hname, eng_act):
            # pairs live in ONE tile so downstream combines sync once
            h = pp.tile([128, QT], f32, name=hname)
            pa = pp.tile([128, (J // 2) * QT], f32, name=f"{hname}_ps")
            # emit pairs in DECREASING i: combines consume high i first
            if eng_act:
                nc.scalar.activation(out=h, in_=a_t,
                                     func=mybir.ActivationFunctionType.Identity,
                                     bias=col(J), scale=0.0)
                for i in range(J // 2 - 1, -1, -1):
                    nc.scalar.activation(
                        out=pa[:, i * QT:(i + 1) * QT], in_=a_t,
                        func=mybir.ActivationFunctionType.Identity,
                        bias=col(2 * i), scale=col(2 * i + 1))
            else:
                nc.vector.tensor_scalar(out=h, in0=a_t, scalar1=0.0,
                                        scalar2=col(J),
                                        op0=mybir.AluOpType.mult,
                                        op1=mybir.AluOpType.add)
                for i in range(J // 2 - 1, -1, -1):
                    nc.vector.tensor_scalar(out=pa[:, i * QT:(i + 1) * QT],
                                            in0=a_t,
                                            scalar1=col(2 * i + 1),
                                            scalar2=col(2 * i),
                                            op0=mybir.AluOpType.mult,
                                            op1=mybir.AluOpType.add)
            return h, pa

        hd, pd = make_pairs(dcol, "hd", ACT_PAIRS)
        hn, pn = make_pairs(ncol, "hn", False)
        for i in range(J // 2 - 1, -1, -1):
            nc.vector.tensor_mul(hd, hd, a2)
            nc.vector.tensor_add(hd, hd, pd[:, i * QT:(i + 1) * QT])
            nc.vector.tensor_mul(hn, hn, a2)
            nc.vector.tensor_add(hn, hn, pn[:, i * QT:(i + 1) * QT])

        r = sb.tile([128, QT], f32)
        nc.vector.reciprocal(out=r, in_=hd)
        out_t = sb.tile([128, QT], f32)
        nc.vector.tensor_mul(out_t, hn, r)
        nc.vector.tensor_scalar_mul(out=out_t, in0=out_t, scalar1=DN[:, 31:32])
        nc.scalar.dma_start(out=out, in_=out_t)

    return nc


def _build_moment_v5(nc):
    """v5: single matmul with all-ones [128,128] stationary both reduces the
    moment partials across partitions AND broadcasts them to every partition;
    factorial scaling uses host-pre-broadcast constant columns; c,s computed
    redundantly per-partition from a broadcast-DMA of the weights."""
    f32 = F32
    QT = QPC // 128
    assert J == 14
    # xin cols: xk(32) | xq(16); cst cols: w(64) | factD(15) | factN(15) | D0(1)
    FD0, FN0, D00 = 64, 79, 94
    xin = nc.dram_tensor("xin", [128, KT + QT], f32, kind="ExternalInput").ap()
    cst = nc.dram_tensor("cst", [128, 95], f32, kind="ExternalInput").ap()
    out = nc.dram_tensor("out", [128, QT], f32, kind="ExternalOutput").ap()

    with tile.TileContext(nc) as tc, ExitStack() as ctx:
        sb = ctx.enter_context(tc.tile_pool(name="sb", bufs=1))
        ps = ctx.enter_context(tc.tile_pool(name="ps", bufs=1, space="PSUM"))

        xin_sb = sb.tile([128, KT + QT], f32)
        nc.sync.dma_start(out=xin_sb, in_=xin)
        cst_sb = sb.tile([128, 95], f32)
        nc.scalar.dma_start(out=cst_sb, in_=cst)
        wbc = cst_sb[:, 0:64]
        xk = xin_sb[:, 0:KT]
        xq_t = xin_sb[:, KT:KT + QT]

        warm = sb.tile([1, 1], f32)
        nc.vector.memset(warm, 0.0)
        nc.scalar.add(out=warm, in_=warm, add=0.0)  # absorb ACT table load
        ones128 = sb.tile([128, 128], f32)
        nc.vector.memset(ones128, 1.0)

        # ---- phase 1: odd powers on DVE, even powers on ACT ----
        A = sb.tile([128, 7 * KT], f32)   # x^3..x^15
        Bt = sb.tile([128, 7 * KT], f32)  # x^2..x^14
        x2d = sb.tile([128, KT], f32)
        nc.vector.tensor_mul(x2d, xk, xk)
        nc.vector.tensor_mul(A[:, 0:KT], x2d, xk)
        for i in range(1, 7):
            nc.vector.tensor_mul(A[:, i * KT:(i + 1) * KT],
                                 A[:, (i - 1) * KT:i * KT], x2d)

        # ---- c, s per-partition (replicated); after the chain on DVE ----
        prod = sb.tile([128, 32], f32)
        nc.vector.tensor_mul(prod[:, 0:16], wbc[:, 0:16], wbc[:, 16:32])
        nc.vector.tensor_mul(prod[:, 16:32], wbc[:, 32:48], wbc[:, 48:64])
        cs = sb.tile([128, 2], f32)
        nc.vector.reduce_sum(cs[:, 0:1], prod[:, 0:16], axis=mybir.AxisListType.X)
        nc.vector.reduce_sum(cs[:, 1:2], prod[:, 16:32], axis=mybir.AxisListType.X)
        nc.vector.tensor_scalar_mul(out=cs[:, 0:1], in0=cs[:, 0:1], scalar1=0.25)
        a_t = sb.tile([128, QT], f32)
        nc.vector.tensor_scalar_mul(out=a_t, in0=xq_t, scalar1=cs[:, 0:1])
        a2 = sb.tile([128, QT], f32)
        nc.vector.tensor_mul(a2, a_t, a_t)
        sq = mybir.ActivationFunctionType.Square
        nc.scalar.activation(out=Bt[:, 0:KT], in_=xk, func=sq)
        nc.scalar.activation(out=Bt[:, KT:2 * KT], in_=Bt[:, 0:KT], func=sq)
        nc.scalar.activation(out=Bt[:, 2 * KT:3 * KT], in_=A[:, 0:KT], func=sq)
        nc.scalar.activation(out=Bt[:, 3 * KT:4 * KT], in_=Bt[:, KT:2 * KT], func=sq)
        nc.scalar.activation(out=Bt[:, 4 * KT:5 * KT], in_=A[:, KT:2 * KT], func=sq)
        nc.scalar.activation(out=Bt[:, 5 * KT:6 * KT], in_=Bt[:, 2 * KT:3 * KT], func=sq)
        nc.scalar.activation(out=Bt[:, 6 * KT:7 * KT], in_=A[:, 2 * KT:3 * KT], func=sq)
        U = sb.tile([128, 15], f32)       # [M1 | M3..M15 | M2..M14] partials
        nc.vector.reduce_sum(U[:, 0:1], xk, axis=mybir.AxisListType.X)
        nc.vector.reduce_sum(U[:, 1:8], A.rearrange("p (j f) -> p j f", f=KT),
                             axis=mybir.AxisListType.X)
        nc.vector.reduce_sum(U[:, 8:15], Bt.rearrange("p (j f) -> p j f", f=KT),
                             axis=mybir.AxisListType.X)

        # ---- reduce + broadcast in one matmul ----
        ps_mbc = ps.tile([128, 15], f32)
        nc.tensor.matmul(ps_mbc, lhsT=ones128, rhs=U, start=True, stop=True)
        Dc = sb.tile([128, 15], f32)
        nc.vector.tensor_mul(Dc, ps_mbc, cst_sb[:, FD0:FD0 + 15])
        Nc = sb.tile([128, 15], f32)
        nc.vector.tensor_mul(Nc, ps_mbc, cst_sb[:, FN0:FN0 + 15])
        nc.vector.tensor_scalar_mul(out=Nc, in0=Nc, scalar1=cs[:, 1:2])  # fold s

        def dcol(j):
            if j == 0:
                return cst_sb[:, D00:D00 + 1]
            i = (j - 1) // 2 if j % 2 == 1 else 8 + j // 2 - 1
            return Dc[:, i:i + 1]

        def ncol(j):
            i = j // 2 if j % 2 == 0 else 8 + (j - 1) // 2
            return Nc[:, i:i + 1]

        # ---- phase 2: Estrin (den pairs on ACT, num on DVE) ----
        pp = ctx.enter_context(tc.tile_pool(name="pp", bufs=1))

        def make_pairs(col, hname, eng_act):
            h = pp.tile([128, QT], f32, name=hname)
            pa = pp.tile([128, (J // 2) * QT], f32, name=f"{hname}_ps")
            if eng_act:
                nc.scalar.activation(out=h, in_=a_t,
                                     func=mybir.ActivationFunctionType.Identity,
                                     bias=col(J), scale=0.0)
                for i in range(J // 2 - 1, -1, -1):
                    nc.scalar.activation(
                        out=pa[:, i * QT:(i + 1) * QT], in_=a_t,
                        func=mybir.ActivationFunctionType.Identity,
                        bias=col(2 * i), scale=col(2 * i + 1))
            else:
                nc.vector.tensor_scalar(out=h, in0=a_t, scalar1=0.0,
                                        scalar2=col(J),
                                        op0=mybir.AluOpType.mult,
                                        op1=mybir.AluOpType.add)
                for i in range(J // 2 - 1, -1, -1):
                    nc.vector.tensor_scalar(out=pa[:, i * QT:(i + 1) * QT],
                                            in0=a_t,
                                            scalar1=col(2 * i + 1),
                                            scalar2=col(2 * i),
                                            op0=mybir.AluOpType.mult,
                                            op1=mybir.AluOpType.add)
            return h, pa

        hd, pd = make_pairs(dcol, "hd", True)
        hn, pn = make_pairs(ncol, "hn", False)
        for i in range(J // 2 - 1, -1, -1):
            nc.vector.tensor_mul(hd, hd, a2)
            nc.vector.tensor_add(hd, hd, pd[:, i * QT:(i + 1) * QT])
            nc.vector.tensor_mul(hn, hn, a2)
            nc.vector.tensor_add(hn, hn, pn[:, i * QT:(i + 1) * QT])

        r = sb.tile([128, QT], f32)
        nc.vector.reciprocal(out=r, in_=hd)
        out_t = sb.tile([128, QT], f32)
        nc.vector.tensor_mul(out_t, hn, r)
        nc.scalar.dma_start(out=out, in_=out_t)

    return nc


def _build_moment(nc):
    """Moment/Taylor kernel.

    den(a) = sum_k e^{a x_k} = sum_j (M_j/j!) a^j  with M_j = sum_k x_k^j
    num(a) = sum_k x_k e^{a x_k} = sum_j (M_{j+1}/j!) a^j
    out(q) = s * num(a_q)/den(a_q),  a_q = c*x_q.
    Per core: 2048 queries as [128,16], 4096 keys as [128,32].
    """
    f32 = F32
    xkey = nc.dram_tensor("xkey", [128, KT], f32, kind="ExternalInput").ap()
    xq = nc.dram_tensor("xq", [128, QPC // 128], f32, kind="ExternalInput").ap()
    w = nc.dram_tensor("w", [1, 64], f32, kind="ExternalInput").ap()
    fact = nc.dram_tensor("fact", [1, 32], f32, kind="ExternalInput").ap()
    out = nc.dram_tensor("out", [128, QPC // 128], f32, kind="ExternalOutput").ap()

    NW = 2 * J + 4  # D[0..J] | N[0..J] | c | s

    with tile.TileContext(nc) as tc, ExitStack() as ctx:
        sb = ctx.enter_context(tc.tile_pool(name="sb", bufs=1))
        upool = ctx.enter_context(tc.tile_pool(name="upool", bufs=2))
        ps = ctx.enter_context(tc.tile_pool(name="ps", bufs=1, space="PSUM"))

        w_sb = sb.tile([1, 64], f32)
        nc.sync.dma_start(out=w_sb, in_=w)
        fact_sb = sb.tile([1, 32], f32)
        nc.sync.dma_start(out=fact_sb, in_=fact)
        xk_sb = sb.tile([128, KT], f32)
        nc.sync.dma_start(out=xk_sb, in_=xkey)
        xq_sb = sb.tile([128, QPC // 128], f32)
        nc.sync.dma_start(out=xq_sb, in_=xq)

        # c = 0.25*dot(wq,wk), s = dot(wv,wout) on partition 0
        prod = sb.tile([1, 32], f32)
        nc.vector.tensor_mul(prod[:, 0:16], w_sb[:, 0:16], w_sb[:, 16:32])
        nc.vector.tensor_mul(prod[:, 16:32], w_sb[:, 32:48], w_sb[:, 48:64])
        cs = sb.tile([1, 2], f32)
        nc.vector.reduce_sum(cs[:, 0:1], prod[:, 0:16], axis=mybir.AxisListType.X)
        nc.vector.reduce_sum(cs[:, 1:2], prod[:, 16:32], axis=mybir.AxisListType.X)
        nc.scalar.mul(out=cs[:, 0:1], in_=cs[:, 0:1], mul=0.25)

        # ---- phase 1: per-partition moment partials U[:, j] = sum_f x^j ----
        ones_kt = sb.tile([128, KT], f32)
        nc.vector.memset(ones_kt, 1.0)
        U = sb.tile([128, J + 2], f32)
        nc.vector.memset(U[:, 0:1], float(KT))  # sum of x^0 per partition
        nc.vector.reduce_sum(U[:, 1:2], xk_sb, axis=mybir.AxisListType.X)
        USE_TTR = os.environ.get("ATTN_TTR", "0") == "1"
        uprev = xk_sb
        for j in range(2, J + 2):
            u = upool.tile([128, KT], f32, tag="u")
            if USE_TTR:
                nc.vector.tensor_tensor_reduce(
                    out=u, in0=uprev, in1=xk_sb, scale=1.0, scalar=0.0,
                    op0=mybir.AluOpType.mult, op1=mybir.AluOpType.add,
                    accum_out=U[:, j:j + 1])
            else:
                nc.vector.tensor_mul(u, uprev, xk_sb)
                nc.vector.reduce_sum(U[:, j:j + 1], u, axis=mybir.AxisListType.X)
            uprev = u

        # ---- cross-partition reduce: M_row[0, j] = sum_p U[p, j] ----
        ps_m = ps.tile([1, J + 2], f32)
        nc.tensor.matmul(ps_m, lhsT=ones_kt[:, 0:1], rhs=U, start=True, stop=True)

        # ---- coefficient row: D | N*s? | c | s (partition 0) ----
        row = sb.tile([1, NW], f32)
        nc.vector.tensor_mul(row[:, 0:J + 1], ps_m[:, 0:J + 1], fact_sb[:, 0:J + 1])
        nc.vector.tensor_mul(row[:, J + 1:2 * J + 2], ps_m[:, 1:J + 2],
                             fact_sb[:, 0:J + 1])
        nc.vector.tensor_copy(row[:, 2 * J + 2:NW], cs)

        # ---- broadcast coefficients to all partitions ----
        ones_row = sb.tile([1, 128], f32)
        nc.vector.memset(ones_row, 1.0)
        ps_bc = ps.tile([128, NW], f32)
        nc.tensor.matmul(ps_bc, lhsT=ones_row, rhs=row, start=True, stop=True)
        DN = sb.tile([128, NW], f32)
        nc.scalar.copy(out=DN, in_=ps_bc)

        # ---- phase 2: Horner at a = c*xq ----
        QT = QPC // 128
        a_t = sb.tile([128, QT], f32)
        nc.vector.tensor_scalar_mul(out=a_t, in0=xq_sb,
                                    scalar1=DN[:, 2 * J + 2:2 * J + 3])
        hd = sb.tile([128, QT], f32)
        hn = sb.tile([128, QT], f32)
        nc.vector.tensor_scalar(out=hd, in0=a_t, scalar1=0.0,
                                scalar2=DN[:, J:J + 1],
                                op0=mybir.AluOpType.mult,
                                op1=mybir.AluOpType.add)
        nc.vector.tensor_scalar(out=hn, in0=a_t, scalar1=0.0,
                                scalar2=DN[:, 2 * J + 1:2 * J + 2],
                                op0=mybir.AluOpType.mult,
                                op1=mybir.AluOpType.add)
        for j in range(J - 1, -1, -1):
            nc.vector.tensor_mul(hd, hd, a_t)
            nc.vector.tensor_scalar_add(out=hd, in0=hd, scalar1=DN[:, j:j + 1])
            nc.vector.tensor_mul(hn, hn, a_t)
            nc.vector.tensor_scalar_add(out=hn, in0=hn,
                                        scalar1=DN[:, J + 1 + j:J + 2 + j])

        # ---- out = s * hn / hd ----
        r = sb.tile([128, QT], f32)
        nc.vector.reciprocal(out=r, in_=hd)
        out_t = sb.tile([128, QT], f32)
        nc.vector.tensor_mul(out_t, hn, r)
        nc.vector.tensor_scalar_mul(out=out_t, in0=out_t,
                                    scalar1=DN[:, 2 * J + 3:NW])
        nc.sync.dma_start(out=out, in_=out_t)

    return nc


_CACHE = {}


def _get_nc():
    key = (KERNEL, MM_DTYPE, J, DUAL, os.environ.get("ATTN_PH1", "pp"),
           os.environ.get("ATTN_ACT", "1"))
    if key not in _CACHE:
        ndev = int(os.environ.get("ATTN_NDEV", str(NCORES)))
        nc = bacc.Bacc("TRN2", target_bir_lowering=False, debug=False,
                       num_devices=ndev)
        if KERNEL == "moment5":
            _build_moment_v5(nc)
        elif KERNEL == "moment4":
            _build_moment_v4(nc)
        elif KERNEL == "moment3":
            _build_moment_v3(nc)
        elif KERNEL == "moment2":
            _build_moment_v2(nc)
        elif KERNEL == "moment":
            _build_moment(nc)
        else:
            _build_brute(nc)
        nc.compile()
        _CACHE[key] = nc
    return _CACHE[key]


def _in_maps(x, w_q, w_k, w_v, w_out):
    import math
    w_all = np.concatenate([
        np.asarray(w_q, np.float32).ravel(),
        np.asarray(w_k, np.float32).ravel(),
        np.asarray(w_v, np.float32).ravel(),
        np.asarray(w_out, np.float32).ravel(),
    ]).reshape(1, 64)
    x = np.asarray(x, np.float32)
    fact = np.array([1.0 / math.factorial(j) for j in range(32)],
                    np.float64).astype(np.float32).reshape(1, 32)
    maps = []
    for core in range(NCORES):
        b, h = divmod(core, NCORES // B)
        if KERNEL == "moment5":
            import math as _m
            factD = np.zeros(15, np.float64)
            factN = np.zeros(15, np.float64)
            for i in range(8):
                factD[i] = 0.0 if i == 7 else 1.0 / _m.factorial(2 * i + 1)
                factN[i] = 1.0 / _m.factorial(2 * i)
            for t in range(7):
                factD[8 + t] = 1.0 / _m.factorial(2 * t + 2)
                factN[8 + t] = 1.0 / _m.factorial(2 * t + 1)
            consts = np.concatenate([w_all.ravel(), factD, factN,
                                     [float(N)]]).astype(np.float32)
            xin = np.concatenate([
                x[b].reshape(128, KT),
                x[b, h * QPC:(h + 1) * QPC].reshape(128, QPC // 128),
            ], axis=1)
            maps.append({
                "xin": np.ascontiguousarray(xin.astype(np.float32)),
                "cst": np.ascontiguousarray(np.tile(consts.reshape(1, 95), (128, 1))),
            })
        elif KERNEL in ("moment2", "moment3", "moment4"):
            xin = np.concatenate([
                x[b].reshape(128, KT),
                x[b, h * QPC:(h + 1) * QPC].reshape(128, QPC // 128),
            ], axis=1)
            if KERNEL == "moment4":
                f = fact.ravel()
                fDodd = f[1:15:2]                      # 1/1!,1/3!..1/13!
                fDeven = f[2:16:2]                     # 1/2!..1/14!
                fNeven = f[0:16:2]                     # 1/0!,1/2!..1/14!
                fNodd = f[1:15:2]                      # 1/1!..1/13!
                fs = np.concatenate([fDodd, fDeven, fNeven, fNodd,
                                     np.zeros(3, np.float32)])
                win = np.concatenate([w_all.ravel(), fs]).reshape(1, 96)
            else:
                win = np.concatenate([w_all.ravel(), fact.ravel()]).reshape(1, 96)
            maps.append({
                "xin": np.ascontiguousarray(xin),
                "win": np.ascontiguousarray(win.astype(np.float32)),
            })
        elif KERNEL == "moment":
            maps.append({
                "xq": np.ascontiguousarray(
                    x[b, h * QPC:(h + 1) * QPC].reshape(128, QPC // 128)),
                "xkey": np.ascontiguousarray(x[b].reshape(128, KT)),
                "w": w_all,
                "fact": fact,
            })
        else:
            maps.append({
                "xq": np.ascontiguousarray(x[b, h * QPC:(h + 1) * QPC].reshape(1, QPC)),
                "xk": np.ascontiguousarray(x[b].reshape(KT, 128).T),
                "w": w_all,
            })
    return maps


def run(x, w_q, w_k, w_v, w_out, trace=False):
    global KERNEL
    if KERNEL.startswith("moment"):
        # safety guard: the Taylor path is validated for |score| <= T_GUARD.
        # (scores = c * x_q * x_k; for the target data max |score| ~ 3.97)
        c = float(np.dot(np.asarray(w_q, np.float64).ravel(),
                         np.asarray(w_k, np.float64).ravel())) / 4.0
        tmax = abs(c) * float((np.abs(np.asarray(x)).max(axis=1) ** 2).max())
        if tmax > T_GUARD:
            KERNEL = "brute"
    nc = _get_nc()
    maps = _in_maps(x, w_q, w_k, w_v, w_out)
    res = run_bass_kernel_spmd(nc, maps, list(range(NCORES)), trace=trace)
    y = np.zeros((B, N), np.float32)
    for core in range(NCORES):
        b, h = divmod(core, NCORES // B)
        y[b, h * QPC:(h + 1) * QPC] = res.results[core]["out"].reshape(QPC)
    return y, res


def kernel(x, w_q, w_k, w_v, w_out):
    y, _ = run(x, w_q, w_k, w_v, w_out, trace=False)
    return y


# revision 52
# speedup vs baseline: 1.0009x; 1.0009x over previous
"""Trainium2 Bass kernel for nn_AttentionOperator_43069932044621.

Math: the reference is rank-1 attention on scalar tokens:
  q = x[:,None]*w_q ; k = x[:,None]*w_k ; v = x[:,None]*w_v
  scores[b,n,m] = (q.k)/sqrt(D) = c * x[b,n] * x[b,m],  c = (w_q.w_k)/sqrt(16)/TAU
  out[b,n] = s * (sum_m x_m e^{a_n x_m}) / (sum_m e^{a_n x_m}),
             a_n = c*x[b,n],  s = (w_v.w_out)
Scores are in [-4, 4] for this data, so no softmax max-subtraction is needed
(verified: fp32 without stabilization matches reference to ~6e-7).

Sharding: 8 cores = 4 batches x 2 query-halves. Each core holds its batch's
full key row x[b] (4096) and computes 2048 queries x 4096 keys.

Device algorithm per core (key-partition layout):
  - compute c,s on device from w_q/w_k/w_v/w_out (replicated tiny dots)
  - broadcast queries to all 128 partitions via PE (ones[1,128]^T @ xq)
  - for each key tile j (32 tiles of 128 keys):
      E_j[p,q] = exp(cx_k[p,j] * xq[q])            (one ACT instr, [128,2048])
      psum[0,q] += sum_p 1 * E_j[p,q]   (den)       (PE matmul, stationary [128,2])
      psum[1,q] += sum_p s*x_k * E_j[p,q] (num*s)
  - out[q] = psum[1,q] / psum[0,q]
"""

import os
import numpy as np
from contextlib import ExitStack

import concourse.bass as bass
import concourse.tile as tile
from concourse import bacc, mybir
from concourse.bass_utils import run_bass_kernel_spmd

F32 = mybir.dt.float32
F32R = mybir.dt.float32r

B = 4
N = 4096
NCORES = 8
QPC = N // (NCORES // B)      # 2048 queries per core
KT = N // 128                 # 32 key tiles
CHUNK = 512                   # matmul moving free-dim chunk (one PSUM bank)
NCHUNK = QPC // CHUNK         # 4

# matmul dtype for the reduction over keys:
#   "f32r": full speed (1 cyc/col), ~1e-4 worst-case relative error
#   "f32" : 4x slower on PE, exact fp32
MM_DTYPE = os.environ.get("ATTN_MM_DTYPE", "f32")

# which kernel: "moment4" (Taylor-series in a, O(N*J) work, optimized),
# older variants moment/moment2/moment3, or "brute" (O(N^2) exps, exact
# for any score range -- used as fallback if scores exceed Taylor range)
KERNEL = os.environ.get("ATTN_KERNEL", "moment4")
J = int(os.environ.get("ATTN_J", "14"))  # Taylor degree; validated 1.5e-6 at J=14
DUAL = os.environ.get("ATTN_DUAL", "1") == "1"  # den+num Horner in one [128,32] tile
# max |score| the J=14 Taylor path is trusted for; beyond -> brute fallback
T_GUARD = 4.2


def _build_brute(nc):
    """Emit the SPMD program (same for every core) into nc."""
    xq = nc.dram_tensor("xq", [1, QPC], F32, kind="ExternalInput").ap()
    xk = nc.dram_tensor("xk", [128, KT], F32, kind="ExternalInput").ap()
    w = nc.dram_tensor("w", [1, 64], F32, kind="ExternalInput").ap()
    scratch = nc.dram_tensor("scratch", [2, QPC], F32).ap()
    out = nc.dram_tensor("out", [128, QPC // 128], F32, kind="ExternalOutput").ap()

    with tile.TileContext(nc) as tc, ExitStack() as ctx:
        sb = ctx.enter_context(tc.tile_pool(name="sb", bufs=1))
        epool = ctx.enter_context(tc.tile_pool(name="epool", bufs=3))
        psq = ctx.enter_context(tc.tile_pool(name="psq", bufs=1, space="PSUM"))
        psa = ctx.enter_context(tc.tile_pool(name="psa", bufs=1, space="PSUM"))

        # ---- load inputs ----
        w_bc = sb.tile([128, 64], F32)
        w_bcast_ap = bass.AP(tensor=w.tensor, offset=w.offset,
                             ap=[[0, 128]] + list(w.ap[1:]))
        nc.sync.dma_start(out=w_bc, in_=w_bcast_ap)
        xq_sb = sb.tile([1, QPC], F32)
        nc.sync.dma_start(out=xq_sb, in_=xq)
        xk_sb = sb.tile([128, KT], F32)
        nc.sync.dma_start(out=xk_sb, in_=xk)

        # ---- c = 0.25*dot(wq,wk), s = dot(wv,wout), replicated on all partitions
        prod = sb.tile([128, 32], F32)
        nc.vector.tensor_mul(prod[:, 0:16], w_bc[:, 0:16], w_bc[:, 16:32])
        nc.vector.tensor_mul(prod[:, 16:32], w_bc[:, 32:48], w_bc[:, 48:64])
        cs = sb.tile([128, 2], F32)
        nc.vector.reduce_sum(cs[:, 0:1], prod[:, 0:16], axis=mybir.AxisListType.X)
        nc.vector.reduce_sum(cs[:, 1:2], prod[:, 16:32], axis=mybir.AxisListType.X)
        nc.scalar.mul(out=cs[:, 0:1], in_=cs[:, 0:1], mul=0.25)

        # ---- cx_k and stationary (1 | s*x_k) interleaved columns ----
        mm_dt = F32R if MM_DTYPE == "f32r" else F32
        cxk = sb.tile([128, KT], F32)
        nc.vector.tensor_scalar_mul(out=cxk, in0=xk_sb, scalar1=cs[:, 0:1])
        stat = sb.tile([128, 2 * KT], mm_dt)
        stat3 = stat.rearrange("p (j t) -> p j t", t=2)
        xk3 = xk_sb.rearrange("p (j t) -> p j t", t=1)
        # ones in even columns: (xk*0)+1 — memset can't write f32r
        nc.vector.tensor_scalar(out=stat3[:, :, 0:1], in0=xk3,
                                scalar1=0.0, scalar2=1.0,
                                op0=mybir.AluOpType.mult,
                                op1=mybir.AluOpType.add)
        nc.vector.tensor_scalar_mul(out=stat3[:, :, 1:2], in0=xk3,
                                    scalar1=cs[:, 1:2])

        # ---- broadcast queries to all partitions (PE with ones stationary) ----
        ones_row = sb.tile([1, 128], F32)
        nc.vector.memset(ones_row, 1.0)
        ps_q = psq.tile([128, QPC], F32)
        for cix in range(NCHUNK):
            sl = slice(cix * CHUNK, (cix + 1) * CHUNK)
            nc.tensor.matmul(ps_q[:, sl], lhsT=ones_row,
                             rhs=xq_sb[:, sl], start=True, stop=True)

        # ---- main loop over key tiles ----
        ps_acc = psa.tile([2, QPC], F32)
        for j in range(KT):
            e = epool.tile([128, QPC], mm_dt, tag="e")
            nc.scalar.activation(out=e, in_=ps_q,
                                 func=mybir.ActivationFunctionType.Exp,
                                 scale=cxk[:, j:j + 1])
            for cix in range(NCHUNK):
                sl = slice(cix * CHUNK, (cix + 1) * CHUNK)
                nc.tensor.matmul(ps_acc[:, sl],
                                 lhsT=stat[:, 2 * j:2 * j + 2],
                                 rhs=e[:, sl],
                                 start=(j == 0), stop=(j == KT - 1),
                                 skip_group_check=True)

        # ---- tail: out = psum[1]/psum[0] ----
        cp = sb.tile([2, QPC], F32)
        nc.scalar.copy(out=cp, in_=ps_acc)
        # bounce through DRAM to reshape [2,2048] -> 2 x [128,16]
        nc.sync.dma_start(out=scratch, in_=cp)
        den_t = sb.tile([128, QPC // 128], F32)
        num_t = sb.tile([128, QPC // 128], F32)
        sc128 = scratch.rearrange("r (p f) -> r p f", p=128)
        nc.sync.dma_start(out=den_t, in_=sc128[0])
        nc.sync.dma_start(out=num_t, in_=sc128[1])
        recip = sb.tile([128, QPC // 128], F32)
        nc.vector.reciprocal(out=recip, in_=den_t)
        out_t = sb.tile([128, QPC // 128], F32)
        nc.vector.tensor_mul(out_t, num_t, recip)
        nc.sync.dma_start(out=out, in_=out_t)

    return nc


def _build_moment_v2(nc):
    """Optimized moment/Taylor kernel (see _build_moment docstring).

    Changes vs v1: two fused input DMAs, no ScalarE (no ACT table load),
    moments reduced via PE matmul + one wide DVE reduce instead of per-power
    reduces, den+num Horner fused in one [128, 2*QT] tile using a stride-0
    coefficient view, J=16.
    """
    f32 = F32
    QT = QPC // 128          # 16 queries per partition
    J1 = J + 1               # powers x^1..x^{J+1} -> J+1 slices
    NW = 2 * J + 4           # D0..DJ | N0..NJ | c | s
    xin = nc.dram_tensor("xin", [128, KT + QT], f32, kind="ExternalInput").ap()
    win = nc.dram_tensor("win", [1, 96], f32, kind="ExternalInput").ap()
    out = nc.dram_tensor("out", [128, QT], f32, kind="ExternalOutput").ap()

    with tile.TileContext(nc) as tc, ExitStack() as ctx:
        sb = ctx.enter_context(tc.tile_pool(name="sb", bufs=1))
        ps = ctx.enter_context(tc.tile_pool(name="ps", bufs=1, space="PSUM"))

        dma_in = nc.scalar if os.environ.get("ATTN_DMA", "scalar") == "scalar" else nc.sync
        xin_sb = sb.tile([128, KT + QT], f32)
        dma_in.dma_start(out=xin_sb, in_=xin)
        win_sb = sb.tile([1, 96], f32)
        dma_in.dma_start(out=win_sb, in_=win)
        xk = xin_sb[:, 0:KT]
        xq_t = xin_sb[:, KT:KT + QT]
        fact = win_sb[:, 64:96]

        # c = 0.25*dot(wq,wk), s = dot(wv,wout) on partition 0 (DVE only)
        prod = sb.tile([1, 32], f32)
        nc.vector.tensor_mul(prod[:, 0:16], win_sb[:, 0:16], win_sb[:, 16:32])
        nc.vector.tensor_mul(prod[:, 16:32], win_sb[:, 32:48], win_sb[:, 48:64])
        cs = sb.tile([1, 2], f32)
        nc.vector.reduce_sum(cs[:, 0:1], prod[:, 0:16], axis=mybir.AxisListType.X)
        nc.vector.reduce_sum(cs[:, 1:2], prod[:, 16:32], axis=mybir.AxisListType.X)
        nc.vector.tensor_scalar_mul(out=cs[:, 0:1], in0=cs[:, 0:1], scalar1=0.25)

        # ---- phase 1: powers x^1..x^{J+1} as slices of one tile ----
        u_all = sb.tile([128, J1 * KT], f32)
        nc.vector.tensor_copy(u_all[:, 0:KT], xk)
        for i in range(1, J1):
            nc.vector.tensor_mul(u_all[:, i * KT:(i + 1) * KT],
                                 u_all[:, (i - 1) * KT:i * KT], xk)

        # cross-partition sums via PE; then one wide free-reduce
        ones_col = sb.tile([128, 1], f32)
        nc.vector.memset(ones_col, 1.0)
        ps_m = ps.tile([1, J1 * KT], f32)
        for lo in range(0, J1 * KT, 512):
            hi = min(lo + 512, J1 * KT)
            nc.tensor.matmul(ps_m[:, lo:hi], lhsT=ones_col,
                             rhs=u_all[:, lo:hi], start=True, stop=True)
        M = sb.tile([1, J1], f32)
        nc.vector.reduce_sum(M, ps_m.rearrange("o (j f) -> o j f", f=KT),
                             axis=mybir.AxisListType.X)

        # ---- coefficient row: D | N | c | s on partition 0 ----
        row = sb.tile([1, NW], f32)
        nc.vector.memset(row[:, 0:1], float(N))           # D_0 = M_0 = N keys
        nc.vector.tensor_mul(row[:, 1:J + 1], M[:, 0:J], fact[:, 1:J + 1])
        nc.vector.tensor_mul(row[:, J + 1:2 * J + 2], M[:, 0:J + 1],
                             fact[:, 0:J + 1])
        nc.vector.tensor_copy(row[:, 2 * J + 2:NW], cs)

        # ---- broadcast to all partitions ----
        ones_row = sb.tile([1, 128], f32)
        nc.vector.memset(ones_row, 1.0)
        ps_bc = ps.tile([128, NW], f32)
        nc.tensor.matmul(ps_bc, lhsT=ones_row, rhs=row, start=True, stop=True)
        DN = sb.tile([128, NW], f32)
        nc.vector.tensor_copy(DN, ps_bc)

        def dn_view(j):
            # [128, 2, QT] view: [:, 0, :] = D_j broadcast, [:, 1, :] = N_j
            return bass.AP(tensor=DN.tensor, offset=DN.offset + j,
                           ap=[list(DN.ap[0]), [J + 1, 2], [0, QT]])

        # ---- phase 2: fused dual Horner at a = c*xq ----
        a_dup = sb.tile([128, 2 * QT], f32)
        nc.vector.tensor_scalar_mul(out=a_dup[:, 0:QT], in0=xq_t,
                                    scalar1=DN[:, 2 * J + 2:2 * J + 3])
        nc.vector.tensor_copy(a_dup[:, QT:2 * QT], a_dup[:, 0:QT])
        h = sb.tile([128, 2 * QT], f32)
        h3 = h.rearrange("p (t f) -> p t f", t=2)
        if DUAL:
            nc.vector.tensor_copy(h3, dn_view(J))
            for j in range(J - 1, -1, -1):
                nc.vector.tensor_mul(h, h, a_dup)
                nc.vector.tensor_add(h3, h3, dn_view(j))
        else:
            nc.vector.tensor_scalar(out=h[:, 0:QT], in0=a_dup[:, 0:QT],
                                    scalar1=0.0, scalar2=DN[:, J:J + 1],
                                    op0=mybir.AluOpType.mult,
                                    op1=mybir.AluOpType.add)
            nc.vector.tensor_scalar(out=h[:, QT:2 * QT], in0=a_dup[:, 0:QT],
                                    scalar1=0.0, scalar2=DN[:, 2 * J + 1:2 * J + 2],
                                    op0=mybir.AluOpType.mult,
                                    op1=mybir.AluOpType.add)
            for j in range(J - 1, -1, -1):
                nc.vector.tensor_mul(h, h, a_dup)
                nc.vector.tensor_scalar_add(out=h[:, 0:QT], in0=h[:, 0:QT],
                                            scalar1=DN[:, j:j + 1])
                nc.vector.tensor_scalar_add(out=h[:, QT:2 * QT],
                                            in0=h[:, QT:2 * QT],
                                            scalar1=DN[:, J + 1 + j:J + 2 + j])

        # ---- out = s * num/den ----
        r = sb.tile([128, QT], f32)
        nc.vector.reciprocal(out=r, in_=h[:, 0:QT])
        out_t = sb.tile([128, QT], f32)
        nc.vector.tensor_mul(out_t, h[:, QT:2 * QT], r)
        nc.vector.tensor_scalar_mul(out=out_t, in0=out_t,
                                    scalar1=DN[:, 2 * J + 3:NW])
        nc.sync.dma_start(out=out, in_=out_t)

    return nc


def _build_moment_v3(nc):
    """v3: ping-pong power chain, per-j reduces, tiny PE reduction matmul,
    Estrin evaluation (J even), DVE-only compute + 2 tiny matmuls."""
    f32 = F32
    QT = QPC // 128
    NW = 2 * J + 4
    PH1 = os.environ.get("ATTN_PH1", "pp")  # pp | wide
    assert J % 2 == 0
    xin = nc.dram_tensor("xin", [128, KT + QT], f32, kind="ExternalInput").ap()
    win = nc.dram_tensor("win", [1, 96], f32, kind="ExternalInput").ap()
    out = nc.dram_tensor("out", [128, QT], f32, kind="ExternalOutput").ap()

    with tile.TileContext(nc) as tc, ExitStack() as ctx:
        sb = ctx.enter_context(tc.tile_pool(name="sb", bufs=1))
        up = ctx.enter_context(tc.tile_pool(name="up", bufs=2))
        ps = ctx.enter_context(tc.tile_pool(name="ps", bufs=1, space="PSUM"))

        xin_sb = sb.tile([128, KT + QT], f32)
        nc.scalar.dma_start(out=xin_sb, in_=xin)
        win_sb = sb.tile([1, 96], f32)
        nc.scalar.dma_start(out=win_sb, in_=win)
        xk = xin_sb[:, 0:KT]
        xq_t = xin_sb[:, KT:KT + QT]
        fact = win_sb[:, 64:96]

        # ---- phase 1: moments M_1..M_{J+1}; chain first on DVE ----
        NM = J + 1
        U = sb.tile([128, NM], f32)
        if PH1 == "pp":
            nc.vector.reduce_sum(U[:, 0:1], xk, axis=mybir.AxisListType.X)
            uprev = xk
            for i in range(1, NM):
                u = up.tile([128, KT], f32, tag="u")
                nc.vector.tensor_mul(u, uprev, xk)
                nc.vector.reduce_sum(U[:, i:i + 1], u, axis=mybir.AxisListType.X)
                uprev = u
        else:
            u_all = sb.tile([128, NM * KT], f32)
            nc.vector.tensor_copy(u_all[:, 0:KT], xk)
            for i in range(1, NM):
                nc.vector.tensor_mul(u_all[:, i * KT:(i + 1) * KT],
                                     u_all[:, (i - 1) * KT:i * KT], xk)
            nc.vector.reduce_sum(U, u_all.rearrange("p (j f) -> p j f", f=KT),
                                 axis=mybir.AxisListType.X)

        # cs dots overlap the PE reduction below
        prod = sb.tile([1, 32], f32)
        nc.vector.tensor_mul(prod[:, 0:16], win_sb[:, 0:16], win_sb[:, 16:32])
        nc.vector.tensor_mul(prod[:, 16:32], win_sb[:, 32:48], win_sb[:, 48:64])
        cs = sb.tile([1, 2], f32)
        nc.vector.reduce_sum(cs[:, 0:1], prod[:, 0:16], axis=mybir.AxisListType.X)
        nc.vector.reduce_sum(cs[:, 1:2], prod[:, 16:32], axis=mybir.AxisListType.X)
        nc.vector.tensor_scalar_mul(out=cs[:, 0:1], in0=cs[:, 0:1], scalar1=0.25)

        ones_col = sb.tile([128, 1], f32)
        nc.vector.memset(ones_col, 1.0)
        ps_m = ps.tile([1, NM], f32)
        nc.tensor.matmul(ps_m, lhsT=ones_col, rhs=U, start=True, stop=True)

        # ---- coefficient row ----
        row = sb.tile([1, NW], f32)
        nc.vector.memset(row[:, 0:1], float(N))
        nc.vector.tensor_mul(row[:, 1:J + 1], ps_m[:, 0:J], fact[:, 1:J + 1])
        nc.vector.tensor_mul(row[:, J + 1:2 * J + 2], ps_m[:, 0:J + 1],
                             fact[:, 0:J + 1])
        nc.vector.tensor_copy(row[:, 2 * J + 2:NW], cs)

        ones_row = sb.tile([1, 128], f32)
        nc.vector.memset(ones_row, 1.0)
        ps_bc = ps.tile([128, NW], f32)
        nc.tensor.matmul(ps_bc, lhsT=ones_row, rhs=row, start=True, stop=True)
        DN = sb.tile([128, NW], f32)
        nc.vector.tensor_copy(DN, ps_bc)

        # ---- phase 2: Estrin at a = c*xq for den and num ----
        a_t = sb.tile([128, QT], f32)
        nc.vector.tensor_scalar_mul(out=a_t, in0=xq_t,
                                    scalar1=DN[:, 2 * J + 2:2 * J + 3])
        a2 = sb.tile([128, QT], f32)
        nc.vector.tensor_mul(a2, a_t, a_t)

        pp = ctx.enter_context(tc.tile_pool(name="pp", bufs=1))

        def estrin(coef_off, hname):
            # coefficients C_j at DN[:, coef_off + j]
            h = pp.tile([128, QT], f32, name=hname)
            nc.vector.tensor_scalar(out=h, in0=a_t, scalar1=0.0,
                                    scalar2=DN[:, coef_off + J:coef_off + J + 1],
                                    op0=mybir.AluOpType.mult,
                                    op1=mybir.AluOpType.add)
            ptiles = []
            for i in range(J // 2):
                p = pp.tile([128, QT], f32, name=f"{hname}_p{i}")
                nc.vector.tensor_scalar(
                    out=p, in0=a_t,
                    scalar1=DN[:, coef_off + 2 * i + 1:coef_off + 2 * i + 2],
                    scalar2=DN[:, coef_off + 2 * i:coef_off + 2 * i + 1],
                    op0=mybir.AluOpType.mult, op1=mybir.AluOpType.add)
                ptiles.append(p)
            for i in range(J // 2 - 1, -1, -1):
                nc.vector.tensor_mul(h, h, a2)
                nc.vector.tensor_add(h, h, ptiles[i])
            return h

        hd = estrin(0, "hd")
        hn = estrin(J + 1, "hn")

        r = sb.tile([128, QT], f32)
        nc.vector.reciprocal(out=r, in_=hd)
        out_t = sb.tile([128, QT], f32)
        nc.vector.tensor_mul(out_t, hn, r)
        nc.vector.tensor_scalar_mul(out=out_t, in0=out_t,
                                    scalar1=DN[:, 2 * J + 3:NW])
        nc.sync.dma_start(out=out, in_=out_t)

    return nc


def _col_d(j):
    """Column of D_j in the permuted coefficient row (see _build_moment_v4)."""
    if j == 0:
        return 0
    return 1 + (j - 1) // 2 if j % 2 == 1 else 8 + (j // 2 - 1)


def _col_n(j):
    return 15 + j // 2 if j % 2 == 0 else 23 + (j - 1) // 2


def _build_moment_v4(nc):
    """v4: power chain alternates between two tensors (x^p = x^{p-1} * x with
    odd powers in A, even in B) so no op reads the tensor it writes; moments
    land permuted and the factorial row comes from the host pre-permuted.
    Optionally offloads den-side Estrin pairs to the Scalar engine (ACT
    Identity = in*scale+bias with per-partition APs), warmed by a dummy
    activation at kernel start."""
    f32 = F32
    QT = QPC // 128
    assert J == 14, "v4 layout is hardcoded for J=14"
    NW = 32  # D0 | Dodd(7) | Deven(7) | Neven(8) | Nodd(7) | c | s
    ACT_PAIRS = os.environ.get("ATTN_ACT", "1") == "1"
    xin = nc.dram_tensor("xin", [128, KT + QT], f32, kind="ExternalInput").ap()
    win = nc.dram_tensor("win", [1, 96], f32, kind="ExternalInput").ap()
    out = nc.dram_tensor("out", [128, QT], f32, kind="ExternalOutput").ap()

    with tile.TileContext(nc) as tc, ExitStack() as ctx:
        sb = ctx.enter_context(tc.tile_pool(name="sb", bufs=1))
        ps = ctx.enter_context(tc.tile_pool(name="ps", bufs=1, space="PSUM"))

        xin_sb = sb.tile([128, KT + QT], f32)
        nc.scalar.dma_start(out=xin_sb, in_=xin)
        win_sb = sb.tile([1, 96], f32)
        nc.sync.dma_start(out=win_sb, in_=win)
        xk = xin_sb[:, 0:KT]
        xq_t = xin_sb[:, KT:KT + QT]

        if ACT_PAIRS:
            warm = sb.tile([1, 1], f32)
            nc.vector.memset(warm, 0.0)
            nc.scalar.add(out=warm, in_=warm, add=0.0)  # absorb ACT table load

        # ---- phase 1: odd powers x^3..x^15 on DVE, even x^2..x^14 on ACT ----
        SPLIT_CHAIN = os.environ.get("ATTN_SPLIT", "1") == "1" and ACT_PAIRS
        A = sb.tile([128, 7 * KT], f32)   # x^3, x^5, ..., x^15
        Bt = sb.tile([128, 7 * KT], f32)  # x^2, x^4, ..., x^14
        x2d = sb.tile([128, KT], f32)
        nc.vector.tensor_mul(x2d, xk, xk)
        nc.vector.tensor_mul(A[:, 0:KT], x2d, xk)            # x^3
        for i in range(1, 7):                                 # x^5..x^15
            nc.vector.tensor_mul(A[:, i * KT:(i + 1) * KT],
                                 A[:, (i - 1) * KT:i * KT], x2d)
        if SPLIT_CHAIN:
            sq = mybir.ActivationFunctionType.Square
            nc.scalar.activation(out=Bt[:, 0:KT], in_=xk, func=sq)          # x^2
            nc.scalar.activation(out=Bt[:, KT:2 * KT], in_=Bt[:, 0:KT], func=sq)   # x^4
            nc.scalar.activation(out=Bt[:, 2 * KT:3 * KT], in_=A[:, 0:KT], func=sq)  # x^6
            nc.scalar.activation(out=Bt[:, 3 * KT:4 * KT], in_=Bt[:, KT:2 * KT], func=sq)  # x^8
            nc.scalar.activation(out=Bt[:, 4 * KT:5 * KT], in_=A[:, KT:2 * KT], func=sq)   # x^10
            nc.scalar.activation(out=Bt[:, 5 * KT:6 * KT], in_=Bt[:, 2 * KT:3 * KT], func=sq)  # x^12
            nc.scalar.activation(out=Bt[:, 6 * KT:7 * KT], in_=A[:, 2 * KT:3 * KT], func=sq)   # x^14
        else:
            nc.vector.tensor_copy(Bt[:, 0:KT], x2d)           # x^2
            for i in range(1, 7):
                nc.vector.tensor_mul(Bt[:, i * KT:(i + 1) * KT],
                                     Bt[:, (i - 1) * KT:i * KT], x2d)
        U = sb.tile([128, 15], f32)
        nc.vector.reduce_sum(U[:, 0:1], xk, axis=mybir.AxisListType.X)
        nc.vector.reduce_sum(U[:, 1:8], A.rearrange("p (j f) -> p j f", f=KT),
                             axis=mybir.AxisListType.X)
        nc.vector.reduce_sum(U[:, 8:15], Bt.rearrange("p (j f) -> p j f", f=KT),
                             axis=mybir.AxisListType.X)

        # cs dots (overlap PE below)
        prod = sb.tile([1, 32], f32)
        nc.vector.tensor_mul(prod[:, 0:16], win_sb[:, 0:16], win_sb[:, 16:32])
        nc.vector.tensor_mul(prod[:, 16:32], win_sb[:, 32:48], win_sb[:, 48:64])
        cs = sb.tile([1, 2], f32)
        nc.vector.reduce_sum(cs[:, 0:1], prod[:, 0:16], axis=mybir.AxisListType.X)
        nc.vector.reduce_sum(cs[:, 1:2], prod[:, 16:32], axis=mybir.AxisListType.X)
        nc.vector.tensor_scalar_mul(out=cs[:, 0:1], in0=cs[:, 0:1], scalar1=0.25)

        ones_col = sb.tile([128, 1], f32)
        nc.vector.memset(ones_col, 1.0)
        ps_m = ps.tile([1, 15], f32)  # [M1,M3..M15, M2,M4..M14]
        nc.tensor.matmul(ps_m, lhsT=ones_col, rhs=U, start=True, stop=True)

        # ---- coefficient row (permuted layout) ----
        # win factors: 64: fDodd(7)=1/1!,1/3!..1/13!; 71: fDeven(7)=1/2!..1/14!
        #              78: fNeven(8)=1/0!,1/2!..1/14!; 86: fNodd(7)=1/1!..1/13!
        row = sb.tile([1, NW], f32)
        nc.vector.memset(row[:, 0:1], float(N))
        nc.vector.tensor_mul(row[:, 1:8], ps_m[:, 0:7], win_sb[:, 64:71])
        nc.vector.tensor_mul(row[:, 8:15], ps_m[:, 8:15], win_sb[:, 71:78])
        nc.vector.tensor_mul(row[:, 15:23], ps_m[:, 0:8], win_sb[:, 78:86])
        nc.vector.tensor_mul(row[:, 23:30], ps_m[:, 8:15], win_sb[:, 86:93])
        nc.vector.tensor_copy(row[:, 30:32], cs)

        ones_row = sb.tile([1, 128], f32)
        nc.vector.memset(ones_row, 1.0)
        ps_bc = ps.tile([128, NW], f32)
        nc.tensor.matmul(ps_bc, lhsT=ones_row, rhs=row, start=True, stop=True)
        DN = sb.tile([128, NW], f32)
        nc.vector.tensor_copy(DN, ps_bc)

        def dcol(j):
            return DN[:, _col_d(j):_col_d(j) + 1]

        def ncol(j):
            return DN[:, _col_n(j):_col_n(j) + 1]

        # ---- phase 2: Estrin; den pairs on ACT (parallel), num on DVE ----
        a_t = sb.tile([128, QT], f32)
        nc.vector.tensor_scalar_mul(out=a_t, in0=xq_t, scalar1=DN[:, 30:31])
        a2 = sb.tile([128, QT], f32)
        nc.vector.tensor_mul(a2, a_t, a_t)

        pp = ctx.enter_context(tc.tile_pool(name="pp", bufs=1))

        def make_pairs(col, hname, eng_act):
            # pairs live in ONE tile so downstream combines sync once
            h = pp.tile([128, QT], f32, name=hname)
            pa = pp.tile([128, (J // 2) * QT], f32, name=f"{hname}_ps")
            # emit pairs in DECREASING i: combines consume high i first
            if eng_act:
                nc.scalar.activation(out=h, in_=a_t,
                                     func=mybir.ActivationFunctionType.Identity,
                                     bias=col(J), scale=0.0)
                for i in range(J // 2 - 1, -1, -1):
                    nc.scalar.activation(
                        out=pa[:, i * QT:(i + 1) * QT], in_=a_t,
                        func=mybir.ActivationFunctionType.Identity,
                        bias=col(2 * i), scale=col(2 * i + 1))
            else:
                nc.vector.tensor_scalar(out=h, in0=a_t, scalar1=0.0,
                                        scalar2=col(J),
                                        op0=mybir.AluOpType.mult,
                                        op1=mybir.AluOpType.add)
                for i in range(J // 2 - 1, -1, -1):
                    nc.vector.tensor_scalar(out=pa[:, i * QT:(i + 1) * QT],
                                            in0=a_t,
                                            scalar1=col(2 * i + 1),
                                            scalar2=col(2 * i),
                                            op0=mybir.AluOpType.mult,
                                            op1=mybir.AluOpType.add)
            return h, pa

        hd, pd = make_pairs(dcol, "hd", ACT_PAIRS)
        hn, pn = make_pairs(ncol, "hn", False)
        for i in range(J // 2 - 1, -1, -1):
            nc.vector.tensor_mul(hd, hd, a2)
            nc.vector.tensor_add(hd, hd, pd[:, i * QT:(i + 1) * QT])
            nc.vector.tensor_mul(hn, hn, a2)
            nc.vector.tensor_add(hn, hn, pn[:, i * QT:(i + 1) * QT])

        r = sb.tile([128, QT], f32)
        nc.vector.reciprocal(out=r, in_=hd)
        out_t = sb.tile([128, QT], f32)
        nc.vector.tensor_mul(out_t, hn, r)
        nc.vector.tensor_scalar_mul(out=out_t, in0=out_t, scalar1=DN[:, 31:32])
        nc.scalar.dma_start(out=out, in_=out_t)

    return nc


def _build_moment_v5(nc):
    """v5: single matmul with all-ones [128,128] stationary both reduces the
    moment partials across partitions AND broadcasts them to every partition;
    factorial scaling uses host-pre-broadcast constant columns; c,s computed
    redundantly per-partition from a broadcast-DMA of the weights."""
    f32 = F32
    QT = QPC // 128
    assert J == 14
    # xin cols: xk(32) | xq(16); cst cols: w(64) | factD(15) | factN(15) | D0(1)
    FD0, FN0, D00 = 64, 79, 94
    xin = nc.dram_tensor("xin", [128, KT + QT], f32, kind="ExternalInput").ap()
    cst = nc.dram_tensor("cst", [128, 95], f32, kind="ExternalInput").ap()
    out = nc.dram_tensor("out", [128, QT], f32, kind="ExternalOutput").ap()

    with tile.TileContext(nc) as tc, ExitStack() as ctx:
        sb = ctx.enter_context(tc.tile_pool(name="sb", bufs=1))
        ps = ctx.enter_context(tc.tile_pool(name="ps", bufs=1, space="PSUM"))

        xin_sb = sb.tile([128, KT + QT], f32)
        nc.sync.dma_start(out=xin_sb, in_=xin)
        cst_sb = sb.tile([128, 95], f32)
        nc.scalar.dma_start(out=cst_sb, in_=cst)
        wbc = cst_sb[:, 0:64]
        xk = xin_sb[:, 0:KT]
        xq_t = xin_sb[:, KT:KT + QT]

        warm = sb.tile([1, 1], f32)
        nc.vector.memset(warm, 0.0)
        nc.scalar.add(out=warm, in_=warm, add=0.0)  # absorb ACT table load
        ones128 = sb.tile([128, 128], f32)
        nc.vector.memset(ones128, 1.0)

        # ---- phase 1: odd powers on DVE, even powers on ACT ----
        A = sb.tile([128, 7 * KT], f32)   # x^3..x^15
        Bt = sb.tile([128, 7 * KT], f32)  # x^2..x^14
        x2d = sb.tile([128, KT], f32)
        nc.vector.tensor_mul(x2d, xk, xk)
        nc.vector.tensor_mul(A[:, 0:KT], x2d, xk)
        for i in range(1, 7):
            nc.vector.tensor_mul(A[:, i * KT:(i + 1) * KT],
                                 A[:, (i - 1) * KT:i * KT], x2d)

        sq = mybir.ActivationFunctionType.Square
        nc.scalar.activation(out=Bt[:, 0:KT], in_=xk, func=sq)
        nc.scalar.activation(out=Bt[:, KT:2 * KT], in_=Bt[:, 0:KT], func=sq)
        nc.scalar.activation(out=Bt[:, 2 * KT:3 * KT], in_=A[:, 0:KT], func=sq)
        nc.scalar.activation(out=Bt[:, 3 * KT:4 * KT], in_=Bt[:, KT:2 * KT], func=sq)
        nc.scalar.activation(out=Bt[:, 4 * KT:5 * KT], in_=A[:, KT:2 * KT], func=sq)
        nc.scalar.activation(out=Bt[:, 5 * KT:6 * KT], in_=Bt[:, 2 * KT:3 * KT], func=sq)
        nc.scalar.activation(out=Bt[:, 6 * KT:7 * KT], in_=A[:, 2 * KT:3 * KT], func=sq)
        U = sb.tile([128, 15], f32)       # [M1 | M3..M15 | M2..M14] partials
        nc.vector.reduce_sum(U[:, 0:1], xk, axis=mybir.AxisListType.X)
        nc.vector.reduce_sum(U[:, 1:8], A.rearrange("p (j f) -> p j f", f=KT),
                             axis=mybir.AxisListType.X)
        nc.vector.reduce_sum(U[:, 8:15], Bt.rearrange("p (j f) -> p j f", f=KT),
                             axis=mybir.AxisListType.X)

        # ---- c, s per-partition (replicated); after the reduces on DVE ----
        prod = sb.tile([128, 32], f32)
        nc.vector.tensor_mul(prod[:, 0:16], wbc[:, 0:16], wbc[:, 16:32])
        nc.vector.tensor_mul(prod[:, 16:32], wbc[:, 32:48], wbc[:, 48:64])
        cs = sb.tile([128, 2], f32)
        nc.vector.reduce_sum(cs[:, 0:1], prod[:, 0:16], axis=mybir.AxisListType.X)
        nc.vector.reduce_sum(cs[:, 1:2], prod[:, 16:32], axis=mybir.AxisListType.X)
        nc.vector.tensor_scalar_mul(out=cs[:, 0:1], in0=cs[:, 0:1], scalar1=0.25)
        # a = c*xq and a^2 on ACT (runs parallel to DVE)
        a_t = sb.tile([128, QT], f32)
        nc.scalar.activation(out=a_t, in_=xq_t,
                             func=mybir.ActivationFunctionType.Identity,
                             bias=0.0, scale=cs[:, 0:1])
        a2 = sb.tile([128, QT], f32)
        nc.scalar.activation(out=a2, in_=a_t, func=sq)

        # ---- reduce + broadcast in one matmul ----
        ps_mbc = ps.tile([128, 15], f32)
        nc.tensor.matmul(ps_mbc, lhsT=ones128, rhs=U, start=True, stop=True)
        Dc = sb.tile([128, 15], f32)
        nc.vector.tensor_mul(Dc, ps_mbc, cst_sb[:, FD0:FD0 + 15])
        Nc = sb.tile([128, 15], f32)
        nc.vector.tensor_mul(Nc, ps_mbc, cst_sb[:, FN0:FN0 + 15])
        nc.vector.tensor_scalar_mul(out=Nc, in0=Nc, scalar1=cs[:, 1:2])  # fold s

        def dcol(j):
            if j == 0:
                return cst_sb[:, D00:D00 + 1]
            i = (j - 1) // 2 if j % 2 == 1 else 8 + j // 2 - 1
            return Dc[:, i:i + 1]

        def ncol(j):
            i = j // 2 if j % 2 == 0 else 8 + (j - 1) // 2
            return Nc[:, i:i + 1]

        # ---- phase 2: Estrin (den pairs on ACT, num on DVE) ----
        pp = ctx.enter_context(tc.tile_pool(name="pp", bufs=1))

        def make_pairs(col, hname, eng_act):
            h = pp.tile([128, QT], f32, name=hname)
            pa = pp.tile([128, (J // 2) * QT], f32, name=f"{hname}_ps")
            if eng_act:
                nc.scalar.activation(out=h, in_=a_t,
                                     func=mybir.ActivationFunctionType.Identity,
                                     bias=col(J), scale=0.0)
                for i in range(J // 2 - 1, -1, -1):
                    nc.scalar.activation(
                        out=pa[:, i * QT:(i + 1) * QT], in_=a_t,
                        func=mybir.ActivationFunctionType.Identity,
                        bias=col(2 * i), scale=col(2 * i + 1))
            else:
                nc.vector.tensor_scalar(out=h, in0=a_t, scalar1=0.0,
                                        scalar2=col(J),
                                        op0=mybir.AluOpType.mult,
                                        op1=mybir.AluOpType.add)
                for i in range(J // 2 - 1, -1, -1):
                    nc.vector.tensor_scalar(out=pa[:, i * QT:(i + 1) * QT],
                                            in0=a_t,
                                            scalar1=col(2 * i + 1),
                                            scalar2=col(2 * i),
                                            op0=mybir.AluOpType.mult,
                                            op1=mybir.AluOpType.add)
            return h, pa

        hd, pd = make_pairs(dcol, "hd", True)
        hn, pn = make_pairs(ncol, "hn", False)
        for i in range(J // 2 - 1, -1, -1):
            nc.vector.tensor_mul(hd, hd, a2)
            nc.vector.tensor_add(hd, hd, pd[:, i * QT:(i + 1) * QT])
            nc.vector.tensor_mul(hn, hn, a2)
            nc.vector.tensor_add(hn, hn, pn[:, i * QT:(i + 1) * QT])

        r = sb.tile([128, QT], f32)
        nc.vector.reciprocal(out=r, in_=hd)
        out_t = sb.tile([128, QT], f32)
        nc.vector.tensor_mul(out_t, hn, r)
        nc.scalar.dma_start(out=out, in_=out_t)

    return nc


def _build_moment(nc):
    """Moment/Taylor kernel.

    den(a) = sum_k e^{a x_k} = sum_j (M_j/j!) a^j  with M_j = sum_k x_k^j
    num(a) = sum_k x_k e^{a x_k} = sum_j (M_{j+1}/j!) a^j
    out(q) = s * num(a_q)/den(a_q),  a_q = c*x_q.
    Per core: 2048 queries as [128,16], 4096 keys as [128,32].
    """
    f32 = F32
    xkey = nc.dram_tensor("xkey", [128, KT], f32, kind="ExternalInput").ap()
    xq = nc.dram_tensor("xq", [128, QPC // 128], f32, kind="ExternalInput").ap()
    w = nc.dram_tensor("w", [1, 64], f32, kind="ExternalInput").ap()
    fact = nc.dram_tensor("fact", [1, 32], f32, kind="ExternalInput").ap()
    out = nc.dram_tensor("out", [128, QPC // 128], f32, kind="ExternalOutput").ap()

    NW = 2 * J + 4  # D[0..J] | N[0..J] | c | s

    with tile.TileContext(nc) as tc, ExitStack() as ctx:
        sb = ctx.enter_context(tc.tile_pool(name="sb", bufs=1))
        upool = ctx.enter_context(tc.tile_pool(name="upool", bufs=2))
        ps = ctx.enter_context(tc.tile_pool(name="ps", bufs=1, space="PSUM"))

        w_sb = sb.tile([1, 64], f32)
        nc.sync.dma_start(out=w_sb, in_=w)
        fact_sb = sb.tile([1, 32], f32)
        nc.sync.dma_start(out=fact_sb, in_=fact)
        xk_sb = sb.tile([128, KT], f32)
        nc.sync.dma_start(out=xk_sb, in_=xkey)
        xq_sb = sb.tile([128, QPC // 128], f32)
        nc.sync.dma_start(out=xq_sb, in_=xq)

        # c = 0.25*dot(wq,wk), s = dot(wv,wout) on partition 0
        prod = sb.tile([1, 32], f32)
        nc.vector.tensor_mul(prod[:, 0:16], w_sb[:, 0:16], w_sb[:, 16:32])
        nc.vector.tensor_mul(prod[:, 16:32], w_sb[:, 32:48], w_sb[:, 48:64])
        cs = sb.tile([1, 2], f32)
        nc.vector.reduce_sum(cs[:, 0:1], prod[:, 0:16], axis=mybir.AxisListType.X)
        nc.vector.reduce_sum(cs[:, 1:2], prod[:, 16:32], axis=mybir.AxisListType.X)
        nc.scalar.mul(out=cs[:, 0:1], in_=cs[:, 0:1], mul=0.25)

        # ---- phase 1: per-partition moment partials U[:, j] = sum_f x^j ----
        ones_kt = sb.tile([128, KT], f32)
        nc.vector.memset(ones_kt, 1.0)
        U = sb.tile([128, J + 2], f32)
        nc.vector.memset(U[:, 0:1], float(KT))  # sum of x^0 per partition
        nc.vector.reduce_sum(U[:, 1:2], xk_sb, axis=mybir.AxisListType.X)
        USE_TTR = os.environ.get("ATTN_TTR", "0") == "1"
        uprev = xk_sb
        for j in range(2, J + 2):
            u = upool.tile([128, KT], f32, tag="u")
            if USE_TTR:
                nc.vector.tensor_tensor_reduce(
                    out=u, in0=uprev, in1=xk_sb, scale=1.0, scalar=0.0,
                    op0=mybir.AluOpType.mult, op1=mybir.AluOpType.add,
                    accum_out=U[:, j:j + 1])
            else:
                nc.vector.tensor_mul(u, uprev, xk_sb)
                nc.vector.reduce_sum(U[:, j:j + 1], u, axis=mybir.AxisListType.X)
            uprev = u

        # ---- cross-partition reduce: M_row[0, j] = sum_p U[p, j] ----
        ps_m = ps.tile([1, J + 2], f32)
        nc.tensor.matmul(ps_m, lhsT=ones_kt[:, 0:1], rhs=U, start=True, stop=True)

        # ---- coefficient row: D | N*s? | c | s (partition 0) ----
        row = sb.tile([1, NW], f32)
        nc.vector.tensor_mul(row[:, 0:J + 1], ps_m[:, 0:J + 1], fact_sb[:, 0:J + 1])
        nc.vector.tensor_mul(row[:, J + 1:2 * J + 2], ps_m[:, 1:J + 2],
                             fact_sb[:, 0:J + 1])
        nc.vector.tensor_copy(row[:, 2 * J + 2:NW], cs)

        # ---- broadcast coefficients to all partitions ----
        ones_row = sb.tile([1, 128], f32)
        nc.vector.memset(ones_row, 1.0)
        ps_bc = ps.tile([128, NW], f32)
        nc.tensor.matmul(ps_bc, lhsT=ones_row, rhs=row, start=True, stop=True)
        DN = sb.tile([128, NW], f32)
        nc.scalar.copy(out=DN, in_=ps_bc)

        # ---- phase 2: Horner at a = c*xq ----
        QT = QPC // 128
        a_t = sb.tile([128, QT], f32)
        nc.vector.tensor_scalar_mul(out=a_t, in0=xq_sb,
                                    scalar1=DN[:, 2 * J + 2:2 * J + 3])
        hd = sb.tile([128, QT], f32)
        hn = sb.tile([128, QT], f32)
        nc.vector.tensor_scalar(out=hd, in0=a_t, scalar1=0.0,
                                scalar2=DN[:, J:J + 1],
                                op0=mybir.AluOpType.mult,
                                op1=mybir.AluOpType.add)
        nc.vector.tensor_scalar(out=hn, in0=a_t, scalar1=0.0,
                                scalar2=DN[:, 2 * J + 1:2 * J + 2],
                                op0=mybir.AluOpType.mult,
                                op1=mybir.AluOpType.add)
        for j in range(J - 1, -1, -1):
            nc.vector.tensor_mul(hd, hd, a_t)
            nc.vector.tensor_scalar_add(out=hd, in0=hd, scalar1=DN[:, j:j + 1])
            nc.vector.tensor_mul(hn, hn, a_t)
            nc.vector.tensor_scalar_add(out=hn, in0=hn,
                                        scalar1=DN[:, J + 1 + j:J + 2 + j])

        # ---- out = s * hn / hd ----
        r = sb.tile([128, QT], f32)
        nc.vector.reciprocal(out=r, in_=hd)
        out_t = sb.tile([128, QT], f32)
        nc.vector.tensor_mul(out_t, hn, r)
        nc.vector.tensor_scalar_mul(out=out_t, in0=out_t,
                                    scalar1=DN[:, 2 * J + 3:NW])
        nc.sync.dma_start(out=out, in_=out_t)

    return nc


_CACHE = {}


def _get_nc():
    key = (KERNEL, MM_DTYPE, J, DUAL, os.environ.get("ATTN_PH1", "pp"),
           os.environ.get("ATTN_ACT", "1"))
    if key not in _CACHE:
        ndev = int(os.environ.get("ATTN_NDEV", str(NCORES)))
        nc = bacc.Bacc("TRN2", target_bir_lowering=False, debug=False,
                       num_devices=ndev)
        if KERNEL == "moment5":
            _build_moment_v5(nc)
        elif KERNEL == "moment4":
            _build_moment_v4(nc)
        elif KERNEL == "moment3":
            _build_moment_v3(nc)
        elif KERNEL == "moment2":
            _build_moment_v2(nc)
        elif KERNEL == "moment":
            _build_moment(nc)
        else:
            _build_brute(nc)
        nc.compile()
        _CACHE[key] = nc
    return _CACHE[key]


def _in_maps(x, w_q, w_k, w_v, w_out):
    import math
    w_all = np.concatenate([
        np.asarray(w_q, np.float32).ravel(),
        np.asarray(w_k, np.float32).ravel(),
        np.asarray(w_v, np.float32).ravel(),
        np.asarray(w_out, np.float32).ravel(),
    ]).reshape(1, 64)
    x = np.asarray(x, np.float32)
    fact = np.array([1.0 / math.factorial(j) for j in range(32)],
                    np.float64).astype(np.float32).reshape(1, 32)
    maps = []
    for core in range(NCORES):
        b, h = divmod(core, NCORES // B)
        if KERNEL == "moment5":
            import math as _m
            factD = np.zeros(15, np.float64)
            factN = np.zeros(15, np.float64)
            for i in range(8):
                factD[i] = 0.0 if i == 7 else 1.0 / _m.factorial(2 * i + 1)
                factN[i] = 1.0 / _m.factorial(2 * i)
            for t in range(7):
                factD[8 + t] = 1.0 / _m.factorial(2 * t + 2)
                factN[8 + t] = 1.0 / _m.factorial(2 * t + 1)
            consts = np.concatenate([w_all.ravel(), factD, factN,
                                     [float(N)]]).astype(np.float32)
            xin = np.concatenate([
                x[b].reshape(128, KT),
                x[b, h * QPC:(h + 1) * QPC].reshape(128, QPC // 128),
            ], axis=1)
            maps.append({
                "xin": np.ascontiguousarray(xin.astype(np.float32)),
                "cst": np.ascontiguousarray(np.tile(consts.reshape(1, 95), (128, 1))),
            })
        elif KERNEL in ("moment2", "moment3", "moment4"):
            xin = np.concatenate([
                x[b].reshape(128, KT),
                x[b, h * QPC:(h + 1) * QPC].reshape(128, QPC // 128),
            ], axis=1)
            if KERNEL == "moment4":
                f = fact.ravel()
                fDodd = f[1:15:2]                      # 1/1!,1/3!..1/13!
                fDeven = f[2:16:2]                     # 1/2!..1/14!
                fNeven = f[0:16:2]                     # 1/0!,1/2!..1/14!
                fNodd = f[1:15:2]                      # 1/1!..1/13!
                fs = np.concatenate([fDodd, fDeven, fNeven, fNodd,
                                     np.zeros(3, np.float32)])
                win = np.concatenate([w_all.ravel(), fs]).reshape(1, 96)
            else:
                win = np.concatenate([w_all.ravel(), fact.ravel()]).reshape(1, 96)
            maps.append({
                "xin": np.ascontiguousarray(xin),
                "win": np.ascontiguousarray(win.astype(np.float32)),
            })
        elif KERNEL == "moment":
            maps.append({
                "xq": np.ascontiguousarray(
                    x[b, h * QPC:(h + 1) * QPC].reshape(128, QPC // 128)),
                "xkey": np.ascontiguousarray(x[b].reshape(128, KT)),
                "w": w_all,
                "fact": fact,
            })
        else:
            maps.append({
                "xq": np.ascontiguousarray(x[b, h * QPC:(h + 1) * QPC].reshape(1, QPC)),
                "xk": np.ascontiguousarray(x[b].reshape(KT, 128).T),
                "w": w_all,
            })
    return maps


def run(x, w_q, w_k, w_v, w_out, trace=False):
    global KERNEL
    if KERNEL.startswith("moment"):
        # safety guard: the Taylor path is validated for |score| <= T_GUARD.
        # (scores = c * x_q * x_k; for the target data max |score| ~ 3.97)
        c = float(np.dot(np.asarray(w_q, np.float64).ravel(),
                         np.asarray(w_k, np.float64).ravel())) / 4.0
        tmax = abs(c) * float((np.abs(np.asarray(x)).max(axis=1) ** 2).max())
        if tmax > T_GUARD:
            KERNEL = "brute"
    nc = _get_nc()
    maps = _in_maps(x, w_q, w_k, w_v, w_out)
    res = run_bass_kernel_spmd(nc, maps, list(range(NCORES)), trace=trace)
    y = np.zeros((B, N), np.float32)
    for core in range(NCORES):
        b, h = divmod(core, NCORES // B)
        y[b, h * QPC:(h + 1) * QPC] = res.results[core]["out"].reshape(QPC)
    return y, res


def kernel(x, w_q, w_k, w_v, w_out):
    y, _ = run(x, w_q, w_k, w_v, w_out, trace=False)
    return y


# revision 54
# speedup vs baseline: 1.0220x; 1.0211x over previous
"""Trainium2 Bass kernel for nn_AttentionOperator_43069932044621.

Math: the reference is rank-1 attention on scalar tokens:
  q = x[:,None]*w_q ; k = x[:,None]*w_k ; v = x[:,None]*w_v
  scores[b,n,m] = (q.k)/sqrt(D) = c * x[b,n] * x[b,m],  c = (w_q.w_k)/sqrt(16)/TAU
  out[b,n] = s * (sum_m x_m e^{a_n x_m}) / (sum_m e^{a_n x_m}),
             a_n = c*x[b,n],  s = (w_v.w_out)
Scores are in [-4, 4] for this data, so no softmax max-subtraction is needed
(verified: fp32 without stabilization matches reference to ~6e-7).

Sharding: 8 cores = 4 batches x 2 query-halves. Each core holds its batch's
full key row x[b] (4096) and computes 2048 queries x 4096 keys.

Device algorithm per core (key-partition layout):
  - compute c,s on device from w_q/w_k/w_v/w_out (replicated tiny dots)
  - broadcast queries to all 128 partitions via PE (ones[1,128]^T @ xq)
  - for each key tile j (32 tiles of 128 keys):
      E_j[p,q] = exp(cx_k[p,j] * xq[q])            (one ACT instr, [128,2048])
      psum[0,q] += sum_p 1 * E_j[p,q]   (den)       (PE matmul, stationary [128,2])
      psum[1,q] += sum_p s*x_k * E_j[p,q] (num*s)
  - out[q] = psum[1,q] / psum[0,q]
"""

import os
import numpy as np
from contextlib import ExitStack

import concourse.bass as bass
import concourse.tile as tile
from concourse import bacc, mybir
from concourse.bass_utils import run_bass_kernel_spmd

F32 = mybir.dt.float32
F32R = mybir.dt.float32r

B = 4
N = 4096
NCORES = 8
QPC = N // (NCORES // B)      # 2048 queries per core
KT = N // 128                 # 32 key tiles
CHUNK = 512                   # matmul moving free-dim chunk (one PSUM bank)
NCHUNK = QPC // CHUNK         # 4

# matmul dtype for the reduction over keys:
#   "f32r": full speed (1 cyc/col), ~1e-4 worst-case relative error
#   "f32" : 4x slower on PE, exact fp32
MM_DTYPE = os.environ.get("ATTN_MM_DTYPE", "f32")

# which kernel: "moment4" (Taylor-series in a, O(N*J) work, optimized),
# older variants moment/moment2/moment3, or "brute" (O(N^2) exps, exact
# for any score range -- used as fallback if scores exceed Taylor range)
KERNEL = os.environ.get("ATTN_KERNEL", "moment5")
J = int(os.environ.get("ATTN_J", "14"))  # Taylor degree; validated 1.5e-6 at J=14
DUAL = os.environ.get("ATTN_DUAL", "1") == "1"  # den+num Horner in one [128,32] tile
# max |score| the J=14 Taylor path is trusted for; beyond -> brute fallback
T_GUARD = 4.2


def _build_brute(nc):
    """Emit the SPMD program (same for every core) into nc."""
    xq = nc.dram_tensor("xq", [1, QPC], F32, kind="ExternalInput").ap()
    xk = nc.dram_tensor("xk", [128, KT], F32, kind="ExternalInput").ap()
    w = nc.dram_tensor("w", [1, 64], F32, kind="ExternalInput").ap()
    scratch = nc.dram_tensor("scratch", [2, QPC], F32).ap()
    out = nc.dram_tensor("out", [128, QPC // 128], F32, kind="ExternalOutput").ap()

    with tile.TileContext(nc) as tc, ExitStack() as ctx:
        sb = ctx.enter_context(tc.tile_pool(name="sb", bufs=1))
        epool = ctx.enter_context(tc.tile_pool(name="epool", bufs=3))
        psq = ctx.enter_context(tc.tile_pool(name="psq", bufs=1, space="PSUM"))
        psa = ctx.enter_context(tc.tile_pool(name="psa", bufs=1, space="PSUM"))

        # ---- load inputs ----
        w_bc = sb.tile([128, 64], F32)
        w_bcast_ap = bass.AP(tensor=w.tensor, offset=w.offset,
                             ap=[[0, 128]] + list(w.ap[1:]))
        nc.sync.dma_start(out=w_bc, in_=w_bcast_ap)
        xq_sb = sb.tile([1, QPC], F32)
        nc.sync.dma_start(out=xq_sb, in_=xq)
        xk_sb = sb.tile([128, KT], F32)
        nc.sync.dma_start(out=xk_sb, in_=xk)

        # ---- c = 0.25*dot(wq,wk), s = dot(wv,wout), replicated on all partitions
        prod = sb.tile([128, 32], F32)
        nc.vector.tensor_mul(prod[:, 0:16], w_bc[:, 0:16], w_bc[:, 16:32])
        nc.vector.tensor_mul(prod[:, 16:32], w_bc[:, 32:48], w_bc[:, 48:64])
        cs = sb.tile([128, 2], F32)
        nc.vector.reduce_sum(cs[:, 0:1], prod[:, 0:16], axis=mybir.AxisListType.X)
        nc.vector.reduce_sum(cs[:, 1:2], prod[:, 16:32], axis=mybir.AxisListType.X)
        nc.scalar.mul(out=cs[:, 0:1], in_=cs[:, 0:1], mul=0.25)

        # ---- cx_k and stationary (1 | s*x_k) interleaved columns ----
        mm_dt = F32R if MM_DTYPE == "f32r" else F32
        cxk = sb.tile([128, KT], F32)
        nc.vector.tensor_scalar_mul(out=cxk, in0=xk_sb, scalar1=cs[:, 0:1])
        stat = sb.tile([128, 2 * KT], mm_dt)
        stat3 = stat.rearrange("p (j t) -> p j t", t=2)
        xk3 = xk_sb.rearrange("p (j t) -> p j t", t=1)
        # ones in even columns: (xk*0)+1 — memset can't write f32r
        nc.vector.tensor_scalar(out=stat3[:, :, 0:1], in0=xk3,
                                scalar1=0.0, scalar2=1.0,
                                op0=mybir.AluOpType.mult,
                                op1=mybir.AluOpType.add)
        nc.vector.tensor_scalar_mul(out=stat3[:, :, 1:2], in0=xk3,
                                    scalar1=cs[:, 1:2])

        # ---- broadcast queries to all partitions (PE with ones stationary) ----
        ones_row = sb.tile([1, 128], F32)
        nc.vector.memset(ones_row, 1.0)
        ps_q = psq.tile([128, QPC], F32)
        for cix in range(NCHUNK):
            sl = slice(cix * CHUNK, (cix + 1) * CHUNK)
            nc.tensor.matmul(ps_q[:, sl], lhsT=ones_row,
                             rhs=xq_sb[:, sl], start=True, stop=True)

        # ---- main loop over key tiles ----
        ps_acc = psa.tile([2, QPC], F32)
        for j in range(KT):
            e = epool.tile([128, QPC], mm_dt, tag="e")
            nc.scalar.activation(out=e, in_=ps_q,
                                 func=mybir.ActivationFunctionType.Exp,
                                 scale=cxk[:, j:j + 1])
            for cix in range(NCHUNK):
                sl = slice(cix * CHUNK, (cix + 1) * CHUNK)
                nc.tensor.matmul(ps_acc[:, sl],
                                 lhsT=stat[:, 2 * j:2 * j + 2],
                                 rhs=e[:, sl],
                                 start=(j == 0), stop=(j == KT - 1),
                                 skip_group_check=True)

        # ---- tail: out = psum[1]/psum[0] ----
        cp = sb.tile([2, QPC], F32)
        nc.scalar.copy(out=cp, in_=ps_acc)
        # bounce through DRAM to reshape [2,2048] -> 2 x [128,16]
        nc.sync.dma_start(out=scratch, in_=cp)
        den_t = sb.tile([128, QPC // 128], F32)
        num_t = sb.tile([128, QPC // 128], F32)
        sc128 = scratch.rearrange("r (p f) -> r p f", p=128)
        nc.sync.dma_start(out=den_t, in_=sc128[0])
        nc.sync.dma_start(out=num_t, in_=sc128[1])
        recip = sb.tile([128, QPC // 128], F32)
        nc.vector.reciprocal(out=recip, in_=den_t)
        out_t = sb.tile([128, QPC // 128], F32)
        nc.vector.tensor_mul(out_t, num_t, recip)
        nc.sync.dma_start(out=out, in_=out_t)

    return nc


def _build_moment_v2(nc):
    """Optimized moment/Taylor kernel (see _build_moment docstring).

    Changes vs v1: two fused input DMAs, no ScalarE (no ACT table load),
    moments reduced via PE matmul + one wide DVE reduce instead of per-power
    reduces, den+num Horner fused in one [128, 2*QT] tile using a stride-0
    coefficient view, J=16.
    """
    f32 = F32
    QT = QPC // 128          # 16 queries per partition
    J1 = J + 1               # powers x^1..x^{J+1} -> J+1 slices
    NW = 2 * J + 4           # D0..DJ | N0..NJ | c | s
    xin = nc.dram_tensor("xin", [128, KT + QT], f32, kind="ExternalInput").ap()
    win = nc.dram_tensor("win", [1, 96], f32, kind="ExternalInput").ap()
    out = nc.dram_tensor("out", [128, QT], f32, kind="ExternalOutput").ap()

    with tile.TileContext(nc) as tc, ExitStack() as ctx:
        sb = ctx.enter_context(tc.tile_pool(name="sb", bufs=1))
        ps = ctx.enter_context(tc.tile_pool(name="ps", bufs=1, space="PSUM"))

        dma_in = nc.scalar if os.environ.get("ATTN_DMA", "scalar") == "scalar" else nc.sync
        xin_sb = sb.tile([128, KT + QT], f32)
        dma_in.dma_start(out=xin_sb, in_=xin)
        win_sb = sb.tile([1, 96], f32)
        dma_in.dma_start(out=win_sb, in_=win)
        xk = xin_sb[:, 0:KT]
        xq_t = xin_sb[:, KT:KT + QT]
        fact = win_sb[:, 64:96]

        # c = 0.25*dot(wq,wk), s = dot(wv,wout) on partition 0 (DVE only)
        prod = sb.tile([1, 32], f32)
        nc.vector.tensor_mul(prod[:, 0:16], win_sb[:, 0:16], win_sb[:, 16:32])
        nc.vector.tensor_mul(prod[:, 16:32], win_sb[:, 32:48], win_sb[:, 48:64])
        cs = sb.tile([1, 2], f32)
        nc.vector.reduce_sum(cs[:, 0:1], prod[:, 0:16], axis=mybir.AxisListType.X)
        nc.vector.reduce_sum(cs[:, 1:2], prod[:, 16:32], axis=mybir.AxisListType.X)
        nc.vector.tensor_scalar_mul(out=cs[:, 0:1], in0=cs[:, 0:1], scalar1=0.25)

        # ---- phase 1: powers x^1..x^{J+1} as slices of one tile ----
        u_all = sb.tile([128, J1 * KT], f32)
        nc.vector.tensor_copy(u_all[:, 0:KT], xk)
        for i in range(1, J1):
            nc.vector.tensor_mul(u_all[:, i * KT:(i + 1) * KT],
                                 u_all[:, (i - 1) * KT:i * KT], xk)

        # cross-partition sums via PE; then one wide free-reduce
        ones_col = sb.tile([128, 1], f32)
        nc.vector.memset(ones_col, 1.0)
        ps_m = ps.tile([1, J1 * KT], f32)
        for lo in range(0, J1 * KT, 512):
            hi = min(lo + 512, J1 * KT)
            nc.tensor.matmul(ps_m[:, lo:hi], lhsT=ones_col,
                             rhs=u_all[:, lo:hi], start=True, stop=True)
        M = sb.tile([1, J1], f32)
        nc.vector.reduce_sum(M, ps_m.rearrange("o (j f) -> o j f", f=KT),
                             axis=mybir.AxisListType.X)

        # ---- coefficient row: D | N | c | s on partition 0 ----
        row = sb.tile([1, NW], f32)
        nc.vector.memset(row[:, 0:1], float(N))           # D_0 = M_0 = N keys
        nc.vector.tensor_mul(row[:, 1:J + 1], M[:, 0:J], fact[:, 1:J + 1])
        nc.vector.tensor_mul(row[:, J + 1:2 * J + 2], M[:, 0:J + 1],
                             fact[:, 0:J + 1])
        nc.vector.tensor_copy(row[:, 2 * J + 2:NW], cs)

        # ---- broadcast to all partitions ----
        ones_row = sb.tile([1, 128], f32)
        nc.vector.memset(ones_row, 1.0)
        ps_bc = ps.tile([128, NW], f32)
        nc.tensor.matmul(ps_bc, lhsT=ones_row, rhs=row, start=True, stop=True)
        DN = sb.tile([128, NW], f32)
        nc.vector.tensor_copy(DN, ps_bc)

        def dn_view(j):
            # [128, 2, QT] view: [:, 0, :] = D_j broadcast, [:, 1, :] = N_j
            return bass.AP(tensor=DN.tensor, offset=DN.offset + j,
                           ap=[list(DN.ap[0]), [J + 1, 2], [0, QT]])

        # ---- phase 2: fused dual Horner at a = c*xq ----
        a_dup = sb.tile([128, 2 * QT], f32)
        nc.vector.tensor_scalar_mul(out=a_dup[:, 0:QT], in0=xq_t,
                                    scalar1=DN[:, 2 * J + 2:2 * J + 3])
        nc.vector.tensor_copy(a_dup[:, QT:2 * QT], a_dup[:, 0:QT])
        h = sb.tile([128, 2 * QT], f32)
        h3 = h.rearrange("p (t f) -> p t f", t=2)
        if DUAL:
            nc.vector.tensor_copy(h3, dn_view(J))
            for j in range(J - 1, -1, -1):
                nc.vector.tensor_mul(h, h, a_dup)
                nc.vector.tensor_add(h3, h3, dn_view(j))
        else:
            nc.vector.tensor_scalar(out=h[:, 0:QT], in0=a_dup[:, 0:QT],
                                    scalar1=0.0, scalar2=DN[:, J:J + 1],
                                    op0=mybir.AluOpType.mult,
                                    op1=mybir.AluOpType.add)
            nc.vector.tensor_scalar(out=h[:, QT:2 * QT], in0=a_dup[:, 0:QT],
                                    scalar1=0.0, scalar2=DN[:, 2 * J + 1:2 * J + 2],
                                    op0=mybir.AluOpType.mult,
                                    op1=mybir.AluOpType.add)
            for j in range(J - 1, -1, -1):
                nc.vector.tensor_mul(h, h, a_dup)
                nc.vector.tensor_scalar_add(out=h[:, 0:QT], in0=h[:, 0:QT],
                                            scalar1=DN[:, j:j + 1])
                nc.vector.tensor_scalar_add(out=h[:, QT:2 * QT],
                                            in0=h[:, QT:2 * QT],
                                            scalar1=DN[:, J + 1 + j:J + 2 + j])

        # ---- out = s * num/den ----
        r = sb.tile([128, QT], f32)
        nc.vector.reciprocal(out=r, in_=h[:, 0:QT])
        out_t = sb.tile([128, QT], f32)
        nc.vector.tensor_mul(out_t, h[:, QT:2 * QT], r)
        nc.vector.tensor_scalar_mul(out=out_t, in0=out_t,
                                    scalar1=DN[:, 2 * J + 3:NW])
        nc.sync.dma_start(out=out, in_=out_t)

    return nc


def _build_moment_v3(nc):
    """v3: ping-pong power chain, per-j reduces, tiny PE reduction matmul,
    Estrin evaluation (J even), DVE-only compute + 2 tiny matmuls."""
    f32 = F32
    QT = QPC // 128
    NW = 2 * J + 4
    PH1 = os.environ.get("ATTN_PH1", "pp")  # pp | wide
    assert J % 2 == 0
    xin = nc.dram_tensor("xin", [128, KT + QT], f32, kind="ExternalInput").ap()
    win = nc.dram_tensor("win", [1, 96], f32, kind="ExternalInput").ap()
    out = nc.dram_tensor("out", [128, QT], f32, kind="ExternalOutput").ap()

    with tile.TileContext(nc) as tc, ExitStack() as ctx:
        sb = ctx.enter_context(tc.tile_pool(name="sb", bufs=1))
        up = ctx.enter_context(tc.tile_pool(name="up", bufs=2))
        ps = ctx.enter_context(tc.tile_pool(name="ps", bufs=1, space="PSUM"))

        xin_sb = sb.tile([128, KT + QT], f32)
        nc.scalar.dma_start(out=xin_sb, in_=xin)
        win_sb = sb.tile([1, 96], f32)
        nc.scalar.dma_start(out=win_sb, in_=win)
        xk = xin_sb[:, 0:KT]
        xq_t = xin_sb[:, KT:KT + QT]
        fact = win_sb[:, 64:96]

        # ---- phase 1: moments M_1..M_{J+1}; chain first on DVE ----
        NM = J + 1
        U = sb.tile([128, NM], f32)
        if PH1 == "pp":
            nc.vector.reduce_sum(U[:, 0:1], xk, axis=mybir.AxisListType.X)
            uprev = xk
            for i in range(1, NM):
                u = up.tile([128, KT], f32, tag="u")
                nc.vector.tensor_mul(u, uprev, xk)
                nc.vector.reduce_sum(U[:, i:i + 1], u, axis=mybir.AxisListType.X)
                uprev = u
        else:
            u_all = sb.tile([128, NM * KT], f32)
            nc.vector.tensor_copy(u_all[:, 0:KT], xk)
            for i in range(1, NM):
                nc.vector.tensor_mul(u_all[:, i * KT:(i + 1) * KT],
                                     u_all[:, (i - 1) * KT:i * KT], xk)
            nc.vector.reduce_sum(U, u_all.rearrange("p (j f) -> p j f", f=KT),
                                 axis=mybir.AxisListType.X)

        # cs dots overlap the PE reduction below
        prod = sb.tile([1, 32], f32)
        nc.vector.tensor_mul(prod[:, 0:16], win_sb[:, 0:16], win_sb[:, 16:32])
        nc.vector.tensor_mul(prod[:, 16:32], win_sb[:, 32:48], win_sb[:, 48:64])
        cs = sb.tile([1, 2], f32)
        nc.vector.reduce_sum(cs[:, 0:1], prod[:, 0:16], axis=mybir.AxisListType.X)
        nc.vector.reduce_sum(cs[:, 1:2], prod[:, 16:32], axis=mybir.AxisListType.X)
        nc.vector.tensor_scalar_mul(out=cs[:, 0:1], in0=cs[:, 0:1], scalar1=0.25)

        ones_col = sb.tile([128, 1], f32)
        nc.vector.memset(ones_col, 1.0)
        ps_m = ps.tile([1, NM], f32)
        nc.tensor.matmul(ps_m, lhsT=ones_col, rhs=U, start=True, stop=True)

        # ---- coefficient row ----
        row = sb.tile([1, NW], f32)
        nc.vector.memset(row[:, 0:1], float(N))
        nc.vector.tensor_mul(row[:, 1:J + 1], ps_m[:, 0:J], fact[:, 1:J + 1])
        nc.vector.tensor_mul(row[:, J + 1:2 * J + 2], ps_m[:, 0:J + 1],
                             fact[:, 0:J + 1])
        nc.vector.tensor_copy(row[:, 2 * J + 2:NW], cs)

        ones_row = sb.tile([1, 128], f32)
        nc.vector.memset(ones_row, 1.0)
        ps_bc = ps.tile([128, NW], f32)
        nc.tensor.matmul(ps_bc, lhsT=ones_row, rhs=row, start=True, stop=True)
        DN = sb.tile([128, NW], f32)
        nc.vector.tensor_copy(DN, ps_bc)

        # ---- phase 2: Estrin at a = c*xq for den and num ----
        a_t = sb.tile([128, QT], f32)
        nc.vector.tensor_scalar_mul(out=a_t, in0=xq_t,
                                    scalar1=DN[:, 2 * J + 2:2 * J + 3])
        a2 = sb.tile([128, QT], f32)
        nc.vector.tensor_mul(a2, a_t, a_t)

        pp = ctx.enter_context(tc.tile_pool(name="pp", bufs=1))

        def estrin(coef_off, hname):
            # coefficients C_j at DN[:, coef_off + j]
            h = pp.tile([128, QT], f32, name=hname)
            nc.vector.tensor_scalar(out=h, in0=a_t, scalar1=0.0,
                                    scalar2=DN[:, coef_off + J:coef_off + J + 1],
                                    op0=mybir.AluOpType.mult,
                                    op1=mybir.AluOpType.add)
            ptiles = []
            for i in range(J // 2):
                p = pp.tile([128, QT], f32, name=f"{hname}_p{i}")
                nc.vector.tensor_scalar(
                    out=p, in0=a_t,
                    scalar1=DN[:, coef_off + 2 * i + 1:coef_off + 2 * i + 2],
                    scalar2=DN[:, coef_off + 2 * i:coef_off + 2 * i + 1],
                    op0=mybir.AluOpType.mult, op1=mybir.AluOpType.add)
                ptiles.append(p)
            for i in range(J // 2 - 1, -1, -1):
                nc.vector.tensor_mul(h, h, a2)
                nc.vector.tensor_add(h, h, ptiles[i])
            return h

        hd = estrin(0, "hd")
        hn = estrin(J + 1, "hn")

        r = sb.tile([128, QT], f32)
        nc.vector.reciprocal(out=r, in_=hd)
        out_t = sb.tile([128, QT], f32)
        nc.vector.tensor_mul(out_t, hn, r)
        nc.vector.tensor_scalar_mul(out=out_t, in0=out_t,
                                    scalar1=DN[:, 2 * J + 3:NW])
        nc.sync.dma_start(out=out, in_=out_t)

    return nc


def _col_d(j):
    """Column of D_j in the permuted coefficient row (see _build_moment_v4)."""
    if j == 0:
        return 0
    return 1 + (j - 1) // 2 if j % 2 == 1 else 8 + (j // 2 - 1)


def _col_n(j):
    return 15 + j // 2 if j % 2 == 0 else 23 + (j - 1) // 2


def _build_moment_v4(nc):
    """v4: power chain alternates between two tensors (x^p = x^{p-1} * x with
    odd powers in A, even in B) so no op reads the tensor it writes; moments
    land permuted and the factorial row comes from the host pre-permuted.
    Optionally offloads den-side Estrin pairs to the Scalar engine (ACT
    Identity = in*scale+bias with per-partition APs), warmed by a dummy
    activation at kernel start."""
    f32 = F32
    QT = QPC // 128
    assert J == 14, "v4 layout is hardcoded for J=14"
    NW = 32  # D0 | Dodd(7) | Deven(7) | Neven(8) | Nodd(7) | c | s
    ACT_PAIRS = os.environ.get("ATTN_ACT", "1") == "1"
    xin = nc.dram_tensor("xin", [128, KT + QT], f32, kind="ExternalInput").ap()
    win = nc.dram_tensor("win", [1, 96], f32, kind="ExternalInput").ap()
    out = nc.dram_tensor("out", [128, QT], f32, kind="ExternalOutput").ap()

    with tile.TileContext(nc) as tc, ExitStack() as ctx:
        sb = ctx.enter_context(tc.tile_pool(name="sb", bufs=1))
        ps = ctx.enter_context(tc.tile_pool(name="ps", bufs=1, space="PSUM"))

        xin_sb = sb.tile([128, KT + QT], f32)
        nc.scalar.dma_start(out=xin_sb, in_=xin)
        win_sb = sb.tile([1, 96], f32)
        nc.sync.dma_start(out=win_sb, in_=win)
        xk = xin_sb[:, 0:KT]
        xq_t = xin_sb[:, KT:KT + QT]

        if ACT_PAIRS:
            warm = sb.tile([1, 1], f32)
            nc.vector.memset(warm, 0.0)
            nc.scalar.add(out=warm, in_=warm, add=0.0)  # absorb ACT table load

        # ---- phase 1: odd powers x^3..x^15 on DVE, even x^2..x^14 on ACT ----
        SPLIT_CHAIN = os.environ.get("ATTN_SPLIT", "1") == "1" and ACT_PAIRS
        A = sb.tile([128, 7 * KT], f32)   # x^3, x^5, ..., x^15
        Bt = sb.tile([128, 7 * KT], f32)  # x^2, x^4, ..., x^14
        x2d = sb.tile([128, KT], f32)
        nc.vector.tensor_mul(x2d, xk, xk)
        nc.vector.tensor_mul(A[:, 0:KT], x2d, xk)            # x^3
        for i in range(1, 7):                                 # x^5..x^15
            nc.vector.tensor_mul(A[:, i * KT:(i + 1) * KT],
                                 A[:, (i - 1) * KT:i * KT], x2d)
        if SPLIT_CHAIN:
            sq = mybir.ActivationFunctionType.Square
            nc.scalar.activation(out=Bt[:, 0:KT], in_=xk, func=sq)          # x^2
            nc.scalar.activation(out=Bt[:, KT:2 * KT], in_=Bt[:, 0:KT], func=sq)   # x^4
            nc.scalar.activation(out=Bt[:, 2 * KT:3 * KT], in_=A[:, 0:KT], func=sq)  # x^6
            nc.scalar.activation(out=Bt[:, 3 * KT:4 * KT], in_=Bt[:, KT:2 * KT], func=sq)  # x^8
            nc.scalar.activation(out=Bt[:, 4 * KT:5 * KT], in_=A[:, KT:2 * KT], func=sq)   # x^10
            nc.scalar.activation(out=Bt[:, 5 * KT:6 * KT], in_=Bt[:, 2 * KT:3 * KT], func=sq)  # x^12
            nc.scalar.activation(out=Bt[:, 6 * KT:7 * KT], in_=A[:, 2 * KT:3 * KT], func=sq)   # x^14
        else:
            nc.vector.tensor_copy(Bt[:, 0:KT], x2d)           # x^2
            for i in range(1, 7):
                nc.vector.tensor_mul(Bt[:, i * KT:(i + 1) * KT],
                                     Bt[:, (i - 1) * KT:i * KT], x2d)
        U = sb.tile([128, 15], f32)
        nc.vector.reduce_sum(U[:, 0:1], xk, axis=mybir.AxisListType.X)
        nc.vector.reduce_sum(U[:, 1:8], A.rearrange("p (j f) -> p j f", f=KT),
                             axis=mybir.AxisListType.X)
        nc.vector.reduce_sum(U[:, 8:15], Bt.rearrange("p (j f) -> p j f", f=KT),
                             axis=mybir.AxisListType.X)

        # cs dots (overlap PE below)
        prod = sb.tile([1, 32], f32)
        nc.vector.tensor_mul(prod[:, 0:16], win_sb[:, 0:16], win_sb[:, 16:32])
        nc.vector.tensor_mul(prod[:, 16:32], win_sb[:, 32:48], win_sb[:, 48:64])
        cs = sb.tile([1, 2], f32)
        nc.vector.reduce_sum(cs[:, 0:1], prod[:, 0:16], axis=mybir.AxisListType.X)
        nc.vector.reduce_sum(cs[:, 1:2], prod[:, 16:32], axis=mybir.AxisListType.X)
        nc.vector.tensor_scalar_mul(out=cs[:, 0:1], in0=cs[:, 0:1], scalar1=0.25)

        ones_col = sb.tile([128, 1], f32)
        nc.vector.memset(ones_col, 1.0)
        ps_m = ps.tile([1, 15], f32)  # [M1,M3..M15, M2,M4..M14]
        nc.tensor.matmul(ps_m, lhsT=ones_col, rhs=U, start=True, stop=True)

        # ---- coefficient row (permuted layout) ----
        # win factors: 64: fDodd(7)=1/1!,1/3!..1/13!; 71: fDeven(7)=1/2!..1/14!
        #              78: fNeven(8)=1/0!,1/2!..1/14!; 86: fNodd(7)=1/1!..1/13!
        row = sb.tile([1, NW], f32)
        nc.vector.memset(row[:, 0:1], float(N))
        nc.vector.tensor_mul(row[:, 1:8], ps_m[:, 0:7], win_sb[:, 64:71])
        nc.vector.tensor_mul(row[:, 8:15], ps_m[:, 8:15], win_sb[:, 71:78])
        nc.vector.tensor_mul(row[:, 15:23], ps_m[:, 0:8], win_sb[:, 78:86])
        nc.vector.tensor_mul(row[:, 23:30], ps_m[:, 8:15], win_sb[:, 86:93])
        nc.vector.tensor_copy(row[:, 30:32], cs)

        ones_row = sb.tile([1, 128], f32)
        nc.vector.memset(ones_row, 1.0)
        ps_bc = ps.tile([128, NW], f32)
        nc.tensor.matmul(ps_bc, lhsT=ones_row, rhs=row, start=True, stop=True)
        DN = sb.tile([128, NW], f32)
        nc.vector.tensor_copy(DN, ps_bc)

        def dcol(j):
            return DN[:, _col_d(j):_col_d(j) + 1]

        def ncol(j):
            return DN[:, _col_n(j):_col_n(j) + 1]

        # ---- phase 2: Estrin; den pairs on ACT (parallel), num on DVE ----
        a_t = sb.tile([128, QT], f32)
        nc.vector.tensor_scalar_mul(out=a_t, in0=xq_t, scalar1=DN[:, 30:31])
        a2 = sb.tile([128, QT], f32)
        nc.vector.tensor_mul(a2, a_t, a_t)

        pp = ctx.enter_context(tc.tile_pool(name="pp", bufs=1))

        def make_pairs(col, hname, eng_act):
            # pairs live in ONE tile so downstream combines sync once
            h = pp.tile([128, QT], f32, name=hname)
            pa = pp.tile([128, (J // 2) * QT], f32, name=f"{hname}_ps")
            # emit pairs in DECREASING i: combines consume high i first
            if eng_act:
                nc.scalar.activation(out=h, in_=a_t,
                                     func=mybir.ActivationFunctionType.Identity,
                                     bias=col(J), scale=0.0)
                for i in range(J // 2 - 1, -1, -1):
                    nc.scalar.activation(
                        out=pa[:, i * QT:(i + 1) * QT], in_=a_t,
                        func=mybir.ActivationFunctionType.Identity,
                        bias=col(2 * i), scale=col(2 * i + 1))
            else:
                nc.vector.tensor_scalar(out=h, in0=a_t, scalar1=0.0,
                                        scalar2=col(J),
                                        op0=mybir.AluOpType.mult,
                                        op1=mybir.AluOpType.add)
                for i in range(J // 2 - 1, -1, -1):
                    nc.vector.tensor_scalar(out=pa[:, i * QT:(i + 1) * QT],
                                            in0=a_t,
                                            scalar1=col(2 * i + 1),
                                            scalar2=col(2 * i),
                                            op0=mybir.AluOpType.mult,
                                            op1=mybir.AluOpType.add)
            return h, pa

        hd, pd = make_pairs(dcol, "hd", ACT_PAIRS)
        hn, pn = make_pairs(ncol, "hn", False)
        for i in range(J // 2 - 1, -1, -1):
            nc.vector.tensor_mul(hd, hd, a2)
            nc.vector.tensor_add(hd, hd, pd[:, i * QT:(i + 1) * QT])
            nc.vector.tensor_mul(hn, hn, a2)
            nc.vector.tensor_add(hn, hn, pn[:, i * QT:(i + 1) * QT])

        r = sb.tile([128, QT], f32)
        nc.vector.reciprocal(out=r, in_=hd)
        out_t = sb.tile([128, QT], f32)
        nc.vector.tensor_mul(out_t, hn, r)
        nc.vector.tensor_scalar_mul(out=out_t, in0=out_t, scalar1=DN[:, 31:32])
        nc.scalar.dma_start(out=out, in_=out_t)

    return nc


def _build_moment_v5(nc):
    """v5: single matmul with all-ones [128,128] stationary both reduces the
    moment partials across partitions AND broadcasts them to every partition;
    factorial scaling uses host-pre-broadcast constant columns; c,s computed
    redundantly per-partition from a broadcast-DMA of the weights."""
    f32 = F32
    QT = QPC // 128
    assert J == 14
    # xin cols: xk(32) | xq(16); cst cols: w(64) | factD(15) | factN(15) | D0(1)
    FD0, FN0, D00 = 64, 79, 94
    xin = nc.dram_tensor("xin", [128, KT + QT], f32, kind="ExternalInput").ap()
    cst = nc.dram_tensor("cst", [128, 95], f32, kind="ExternalInput").ap()
    out = nc.dram_tensor("out", [128, QT], f32, kind="ExternalOutput").ap()

    with tile.TileContext(nc) as tc, ExitStack() as ctx:
        sb = ctx.enter_context(tc.tile_pool(name="sb", bufs=1))
        ps = ctx.enter_context(tc.tile_pool(name="ps", bufs=1, space="PSUM"))

        xin_sb = sb.tile([128, KT + QT], f32)
        nc.sync.dma_start(out=xin_sb, in_=xin)
        cst_sb = sb.tile([128, 95], f32)
        nc.scalar.dma_start(out=cst_sb, in_=cst)
        wbc = cst_sb[:, 0:64]
        xk = xin_sb[:, 0:KT]
        xq_t = xin_sb[:, KT:KT + QT]

        warm = sb.tile([1, 1], f32)
        nc.vector.memset(warm, 0.0)
        nc.scalar.add(out=warm, in_=warm, add=0.0)  # absorb ACT table load
        ones128 = sb.tile([128, 128], f32)
        nc.vector.memset(ones128, 1.0)

        # ---- phase 1: odd powers on DVE, even powers on ACT ----
        A = sb.tile([128, 7 * KT], f32)   # x^3..x^15
        Bt = sb.tile([128, 7 * KT], f32)  # x^2..x^14
        x2d = sb.tile([128, KT], f32)
        nc.vector.tensor_mul(x2d, xk, xk)
        nc.vector.tensor_mul(A[:, 0:KT], x2d, xk)
        for i in range(1, 7):
            nc.vector.tensor_mul(A[:, i * KT:(i + 1) * KT],
                                 A[:, (i - 1) * KT:i * KT], x2d)

        sq = mybir.ActivationFunctionType.Square
        nc.scalar.activation(out=Bt[:, 0:KT], in_=xk, func=sq)
        nc.scalar.activation(out=Bt[:, KT:2 * KT], in_=Bt[:, 0:KT], func=sq)
        nc.scalar.activation(out=Bt[:, 2 * KT:3 * KT], in_=A[:, 0:KT], func=sq)
        nc.scalar.activation(out=Bt[:, 3 * KT:4 * KT], in_=Bt[:, KT:2 * KT], func=sq)
        nc.scalar.activation(out=Bt[:, 4 * KT:5 * KT], in_=A[:, KT:2 * KT], func=sq)
        nc.scalar.activation(out=Bt[:, 5 * KT:6 * KT], in_=Bt[:, 2 * KT:3 * KT], func=sq)
        nc.scalar.activation(out=Bt[:, 6 * KT:7 * KT], in_=A[:, 2 * KT:3 * KT], func=sq)
        U = sb.tile([128, 15], f32)       # [M1 | M3..M15 | M2..M14] partials
        nc.vector.reduce_sum(U[:, 0:1], xk, axis=mybir.AxisListType.X)
        nc.vector.reduce_sum(U[:, 1:8], A.rearrange("p (j f) -> p j f", f=KT),
                             axis=mybir.AxisListType.X)
        nc.vector.reduce_sum(U[:, 8:15], Bt.rearrange("p (j f) -> p j f", f=KT),
                             axis=mybir.AxisListType.X)

        # ---- c, s per-partition (replicated); after the reduces on DVE ----
        prod = sb.tile([128, 32], f32)
        nc.vector.tensor_mul(prod[:, 0:16], wbc[:, 0:16], wbc[:, 16:32])
        nc.vector.tensor_mul(prod[:, 16:32], wbc[:, 32:48], wbc[:, 48:64])
        cs = sb.tile([128, 2], f32)
        nc.vector.reduce_sum(cs[:, 0:1], prod[:, 0:16], axis=mybir.AxisListType.X)
        nc.vector.reduce_sum(cs[:, 1:2], prod[:, 16:32], axis=mybir.AxisListType.X)
        nc.vector.tensor_scalar_mul(out=cs[:, 0:1], in0=cs[:, 0:1], scalar1=0.25)
        # a = c*xq and a^2 on ACT (runs parallel to DVE)
        a_t = sb.tile([128, QT], f32)
        nc.scalar.activation(out=a_t, in_=xq_t,
                             func=mybir.ActivationFunctionType.Identity,
                             bias=0.0, scale=cs[:, 0:1])
        a2 = sb.tile([128, QT], f32)
        nc.scalar.activation(out=a2, in_=a_t, func=sq)

        # ---- reduce + broadcast in one matmul ----
        ps_mbc = ps.tile([128, 15], f32)
        nc.tensor.matmul(ps_mbc, lhsT=ones128, rhs=U, start=True, stop=True)
        Dc = sb.tile([128, 15], f32)
        nc.vector.tensor_mul(Dc, ps_mbc, cst_sb[:, FD0:FD0 + 15])
        Nc = sb.tile([128, 15], f32)
        nc.vector.tensor_mul(Nc, ps_mbc, cst_sb[:, FN0:FN0 + 15])
        nc.vector.tensor_scalar_mul(out=Nc, in0=Nc, scalar1=cs[:, 1:2])  # fold s

        def dcol(j):
            if j == 0:
                return cst_sb[:, D00:D00 + 1]
            i = (j - 1) // 2 if j % 2 == 1 else 8 + j // 2 - 1
            return Dc[:, i:i + 1]

        def ncol(j):
            i = j // 2 if j % 2 == 0 else 8 + (j - 1) // 2
            return Nc[:, i:i + 1]

        # ---- phase 2: Estrin (den pairs on ACT, num on DVE) ----
        pp = ctx.enter_context(tc.tile_pool(name="pp", bufs=1))

        def make_pairs(col, hname, eng_act):
            h = pp.tile([128, QT], f32, name=hname)
            pa = pp.tile([128, (J // 2) * QT], f32, name=f"{hname}_ps")
            if eng_act:
                # h init on DVE so the combine chain stays DVE-local
                nc.vector.tensor_scalar(out=h, in0=a_t, scalar1=0.0,
                                        scalar2=col(J),
                                        op0=mybir.AluOpType.mult,
                                        op1=mybir.AluOpType.add)
                for i in range(J // 2 - 1, -1, -1):
                    nc.scalar.activation(
                        out=pa[:, i * QT:(i + 1) * QT], in_=a_t,
                        func=mybir.ActivationFunctionType.Identity,
                        bias=col(2 * i), scale=col(2 * i + 1))
            else:
                nc.vector.tensor_scalar(out=h, in0=a_t, scalar1=0.0,
                                        scalar2=col(J),
                                        op0=mybir.AluOpType.mult,
                                        op1=mybir.AluOpType.add)
                for i in range(J // 2 - 1, -1, -1):
                    nc.vector.tensor_scalar(out=pa[:, i * QT:(i + 1) * QT],
                                            in0=a_t,
                                            scalar1=col(2 * i + 1),
                                            scalar2=col(2 * i),
                                            op0=mybir.AluOpType.mult,
                                            op1=mybir.AluOpType.add)
            return h, pa

        hd, pd = make_pairs(dcol, "hd", True)
        hn, pn = make_pairs(ncol, "hn", False)
        for i in range(J // 2 - 1, -1, -1):
            nc.vector.tensor_mul(hd, hd, a2)
            nc.vector.tensor_add(hd, hd, pd[:, i * QT:(i + 1) * QT])
            nc.vector.tensor_mul(hn, hn, a2)
            nc.vector.tensor_add(hn, hn, pn[:, i * QT:(i + 1) * QT])

        r = sb.tile([128, QT], f32)
        nc.vector.reciprocal(out=r, in_=hd)
        out_t = sb.tile([128, QT], f32)
        nc.vector.tensor_mul(out_t, hn, r)
        nc.scalar.dma_start(out=out, in_=out_t)

    return nc


def _build_moment(nc):
    """Moment/Taylor kernel.

    den(a) = sum_k e^{a x_k} = sum_j (M_j/j!) a^j  with M_j = sum_k x_k^j
    num(a) = sum_k x_k e^{a x_k} = sum_j (M_{j+1}/j!) a^j
    out(q) = s * num(a_q)/den(a_q),  a_q = c*x_q.
    Per core: 2048 queries as [128,16], 4096 keys as [128,32].
    """
    f32 = F32
    xkey = nc.dram_tensor("xkey", [128, KT], f32, kind="ExternalInput").ap()
    xq = nc.dram_tensor("xq", [128, QPC // 128], f32, kind="ExternalInput").ap()
    w = nc.dram_tensor("w", [1, 64], f32, kind="ExternalInput").ap()
    fact = nc.dram_tensor("fact", [1, 32], f32, kind="ExternalInput").ap()
    out = nc.dram_tensor("out", [128, QPC // 128], f32, kind="ExternalOutput").ap()

    NW = 2 * J + 4  # D[0..J] | N[0..J] | c | s

    with tile.TileContext(nc) as tc, ExitStack() as ctx:
        sb = ctx.enter_context(tc.tile_pool(name="sb", bufs=1))
        upool = ctx.enter_context(tc.tile_pool(name="upool", bufs=2))
        ps = ctx.enter_context(tc.tile_pool(name="ps", bufs=1, space="PSUM"))

        w_sb = sb.tile([1, 64], f32)
        nc.sync.dma_start(out=w_sb, in_=w)
        fact_sb = sb.tile([1, 32], f32)
        nc.sync.dma_start(out=fact_sb, in_=fact)
        xk_sb = sb.tile([128, KT], f32)
        nc.sync.dma_start(out=xk_sb, in_=xkey)
        xq_sb = sb.tile([128, QPC // 128], f32)
        nc.sync.dma_start(out=xq_sb, in_=xq)

        # c = 0.25*dot(wq,wk), s = dot(wv,wout) on partition 0
        prod = sb.tile([1, 32], f32)
        nc.vector.tensor_mul(prod[:, 0:16], w_sb[:, 0:16], w_sb[:, 16:32])
        nc.vector.tensor_mul(prod[:, 16:32], w_sb[:, 32:48], w_sb[:, 48:64])
        cs = sb.tile([1, 2], f32)
        nc.vector.reduce_sum(cs[:, 0:1], prod[:, 0:16], axis=mybir.AxisListType.X)
        nc.vector.reduce_sum(cs[:, 1:2], prod[:, 16:32], axis=mybir.AxisListType.X)
        nc.scalar.mul(out=cs[:, 0:1], in_=cs[:, 0:1], mul=0.25)

        # ---- phase 1: per-partition moment partials U[:, j] = sum_f x^j ----
        ones_kt = sb.tile([128, KT], f32)
        nc.vector.memset(ones_kt, 1.0)
        U = sb.tile([128, J + 2], f32)
        nc.vector.memset(U[:, 0:1], float(KT))  # sum of x^0 per partition
        nc.vector.reduce_sum(U[:, 1:2], xk_sb, axis=mybir.AxisListType.X)
        USE_TTR = os.environ.get("ATTN_TTR", "0") == "1"
        uprev = xk_sb
        for j in range(2, J + 2):
            u = upool.tile([128, KT], f32, tag="u")
            if USE_TTR:
                nc.vector.tensor_tensor_reduce(
                    out=u, in0=uprev, in1=xk_sb, scale=1.0, scalar=0.0,
                    op0=mybir.AluOpType.mult, op1=mybir.AluOpType.add,
                    accum_out=U[:, j:j + 1])
            else:
                nc.vector.tensor_mul(u, uprev, xk_sb)
                nc.vector.reduce_sum(U[:, j:j + 1], u, axis=mybir.AxisListType.X)
            uprev = u

        # ---- cross-partition reduce: M_row[0, j] = sum_p U[p, j] ----
        ps_m = ps.tile([1, J + 2], f32)
        nc.tensor.matmul(ps_m, lhsT=ones_kt[:, 0:1], rhs=U, start=True, stop=True)

        # ---- coefficient row: D | N*s? | c | s (partition 0) ----
        row = sb.tile([1, NW], f32)
        nc.vector.tensor_mul(row[:, 0:J + 1], ps_m[:, 0:J + 1], fact_sb[:, 0:J + 1])
        nc.vector.tensor_mul(row[:, J + 1:2 * J + 2], ps_m[:, 1:J + 2],
                             fact_sb[:, 0:J + 1])
        nc.vector.tensor_copy(row[:, 2 * J + 2:NW], cs)

        # ---- broadcast coefficients to all partitions ----
        ones_row = sb.tile([1, 128], f32)
        nc.vector.memset(ones_row, 1.0)
        ps_bc = ps.tile([128, NW], f32)
        nc.tensor.matmul(ps_bc, lhsT=ones_row, rhs=row, start=True, stop=True)
        DN = sb.tile([128, NW], f32)
        nc.scalar.copy(out=DN, in_=ps_bc)

        # ---- phase 2: Horner at a = c*xq ----
        QT = QPC // 128
        a_t = sb.tile([128, QT], f32)
        nc.vector.tensor_scalar_mul(out=a_t, in0=xq_sb,
                                    scalar1=DN[:, 2 * J + 2:2 * J + 3])
        hd = sb.tile([128, QT], f32)
        hn = sb.tile([128, QT], f32)
        nc.vector.tensor_scalar(out=hd, in0=a_t, scalar1=0.0,
                                scalar2=DN[:, J:J + 1],
                                op0=mybir.AluOpType.mult,
                                op1=mybir.AluOpType.add)
        nc.vector.tensor_scalar(out=hn, in0=a_t, scalar1=0.0,
                                scalar2=DN[:, 2 * J + 1:2 * J + 2],
                                op0=mybir.AluOpType.mult,
                                op1=mybir.AluOpType.add)
        for j in range(J - 1, -1, -1):
            nc.vector.tensor_mul(hd, hd, a_t)
            nc.vector.tensor_scalar_add(out=hd, in0=hd, scalar1=DN[:, j:j + 1])
            nc.vector.tensor_mul(hn, hn, a_t)
            nc.vector.tensor_scalar_add(out=hn, in0=hn,
                                        scalar1=DN[:, J + 1 + j:J + 2 + j])

        # ---- out = s * hn / hd ----
        r = sb.tile([128, QT], f32)
        nc.vector.reciprocal(out=r, in_=hd)
        out_t = sb.tile([128, QT], f32)
        nc.vector.tensor_mul(out_t, hn, r)
        nc.vector.tensor_scalar_mul(out=out_t, in0=out_t,
                                    scalar1=DN[:, 2 * J + 3:NW])
        nc.sync.dma_start(out=out, in_=out_t)

    return nc


_CACHE = {}


def _get_nc():
    key = (KERNEL, MM_DTYPE, J, DUAL, os.environ.get("ATTN_PH1", "pp"),
           os.environ.get("ATTN_ACT", "1"))
    if key not in _CACHE:
        ndev = int(os.environ.get("ATTN_NDEV", str(NCORES)))
        nc = bacc.Bacc("TRN2", target_bir_lowering=False, debug=False,
                       num_devices=ndev)
        if KERNEL == "moment5":
            _build_moment_v5(nc)
        elif KERNEL == "moment4":
            _build_moment_v4(nc)
        elif KERNEL == "moment3":
            _build_moment_v3(nc)
        elif KERNEL == "moment2":
            _build_moment_v2(nc)
        elif KERNEL == "moment":
            _build_moment(nc)
        else:
            _build_brute(nc)
        nc.compile()
        _CACHE[key] = nc
    return _CACHE[key]


def _in_maps(x, w_q, w_k, w_v, w_out):
    import math
    w_all = np.concatenate([
        np.asarray(w_q, np.float32).ravel(),
        np.asarray(w_k, np.float32).ravel(),
        np.asarray(w_v, np.float32).ravel(),
        np.asarray(w_out, np.float32).ravel(),
    ]).reshape(1, 64)
    x = np.asarray(x, np.float32)
    fact = np.array([1.0 / math.factorial(j) for j in range(32)],
                    np.float64).astype(np.float32).reshape(1, 32)
    maps = []
    for core in range(NCORES):
        b, h = divmod(core, NCORES // B)
        if KERNEL == "moment5":
            import math as _m
            factD = np.zeros(15, np.float64)
            factN = np.zeros(15, np.float64)
            for i in range(8):
                factD[i] = 0.0 if i == 7 else 1.0 / _m.factorial(2 * i + 1)
                factN[i] = 1.0 / _m.factorial(2 * i)
            for t in range(7):
                factD[8 + t] = 1.0 / _m.factorial(2 * t + 2)
                factN[8 + t] = 1.0 / _m.factorial(2 * t + 1)
            consts = np.concatenate([w_all.ravel(), factD, factN,
                                     [float(N)]]).astype(np.float32)
            xin = np.concatenate([
                x[b].reshape(128, KT),
                x[b, h * QPC:(h + 1) * QPC].reshape(128, QPC // 128),
            ], axis=1)
            maps.append({
                "xin": np.ascontiguousarray(xin.astype(np.float32)),
                "cst": np.ascontiguousarray(np.tile(consts.reshape(1, 95), (128, 1))),
            })
        elif KERNEL in ("moment2", "moment3", "moment4"):
            xin = np.concatenate([
                x[b].reshape(128, KT),
                x[b, h * QPC:(h + 1) * QPC].reshape(128, QPC // 128),
            ], axis=1)
            if KERNEL == "moment4":
                f = fact.ravel()
                fDodd = f[1:15:2]                      # 1/1!,1/3!..1/13!
                fDeven = f[2:16:2]                     # 1/2!..1/14!
                fNeven = f[0:16:2]                     # 1/0!,1/2!..1/14!
                fNodd = f[1:15:2]                      # 1/1!..1/13!
                fs = np.concatenate([fDodd, fDeven, fNeven, fNodd,
                                     np.zeros(3, np.float32)])
                win = np.concatenate([w_all.ravel(), fs]).reshape(1, 96)
            else:
                win = np.concatenate([w_all.ravel(), fact.ravel()]).reshape(1, 96)
            maps.append({
                "xin": np.ascontiguousarray(xin),
                "win": np.ascontiguousarray(win.astype(np.float32)),
            })
        elif KERNEL == "moment":
            maps.append({
                "xq": np.ascontiguousarray(
                    x[b, h * QPC:(h + 1) * QPC].reshape(128, QPC // 128)),
                "xkey": np.ascontiguousarray(x[b].reshape(128, KT)),
                "w": w_all,
                "fact": fact,
            })
        else:
            maps.append({
                "xq": np.ascontiguousarray(x[b, h * QPC:(h + 1) * QPC].reshape(1, QPC)),
                "xk": np.ascontiguousarray(x[b].reshape(KT, 128).T),
                "w": w_all,
            })
    return maps


def run(x, w_q, w_k, w_v, w_out, trace=False):
    global KERNEL
    if KERNEL.startswith("moment"):
        # safety guard: the Taylor path is validated for |score| <= T_GUARD.
        # (scores = c * x_q * x_k; for the target data max |score| ~ 3.97)
        c = float(np.dot(np.asarray(w_q, np.float64).ravel(),
                         np.asarray(w_k, np.float64).ravel())) / 4.0
        tmax = abs(c) * float((np.abs(np.asarray(x)).max(axis=1) ** 2).max())
        if tmax > T_GUARD:
            KERNEL = "brute"
    nc = _get_nc()
    maps = _in_maps(x, w_q, w_k, w_v, w_out)
    res = run_bass_kernel_spmd(nc, maps, list(range(NCORES)), trace=trace)
    y = np.zeros((B, N), np.float32)
    for core in range(NCORES):
        b, h = divmod(core, NCORES // B)
        y[b, h * QPC:(h + 1) * QPC] = res.results[core]["out"].reshape(QPC)
    return y, res


def kernel(x, w_q, w_k, w_v, w_out):
    y, _ = run(x, w_q, w_k, w_v, w_out, trace=False)
    return y
